# revision 16
# baseline (speedup 1.0000x reference)
"""GCN (2-layer, gcn_norm) forward on 8 trn2 NeuronCores.

Math (reference):
    norm = D^-1/2 (A + I) D^-1/2
    h      = relu(norm @ (x @ W1) + b1)
    logits = norm @ (h @ W2) + b2
    out    = log_softmax(logits, axis=1)

Kernel strategy:
  - Never materialize norm:  norm @ v  =  ds * (A_hat @ (ds * v)),  ds = rsqrt(deg).
  - Host stages B = adj.T as per-core column slabs (row slab r of adj, transposed,
    C-contiguous) cast to fp8_e4m3 (0/1 entries are exact). TensorE contracts over
    the partition axis, so B's natural [j, i] layout makes A @ v a clean
    moving-operand matmul with perfect DMA patterns.
  - Each core holds its whole 18.9 MB fp8 slab RESIDENT in SBUF: HBM is read once.
  - 3 TensorE passes over the resident slab, all column-tiled so multiple
    chunk-streams run concurrently in the 128x128 PE array:
      deg    : 4 strips (M=1 ones-vector), hidden under the HBM load
      pass 1 : 4 strips of M=32 ([bf16-hi | bf16-lo] split features)
      pass 2 : 2 strips of M=40 (bf16 features)
    Strip partials are summed by one small fold matmul (f32) per block.
  - Self-loops (the +I) are added exactly via f32 identity-rhs matmuls into the
    same PSUM accumulation group, using the core-local f32 feature chunks.
  - Two tiny AllGathers: w' = ds*x@W1 (f32) and v = (ds*h)@W2 (bf16).
  - log_softmax per 128-row tile after a TensorE transpose back to [node, class].
"""
import numpy as np
import ml_dtypes

from concourse import bacc, mybir, tile
from concourse.bass_utils import run_bass_kernel_spmd

N = 12288
F_IN = 512
HID = 16
CLS = 40
R = 8                 # cores
SLAB = N // R         # 1536 rows per core
P = 128
JC = N // P           # 96 contraction chunks
LC = SLAB // P        # 12 local chunks
NB = SLAB // 512      # 3 i-blocks of 512
F32 = mybir.dt.float32
BF16 = mybir.dt.bfloat16
FP8 = mybir.dt.float8e4

_CACHE = {}


def _build():
    nc = bacc.Bacc("TRN2", target_bir_lowering=False, debug=False,
                   enable_asserts=True, num_devices=R)

    b_in = nc.dram_tensor("b", [N, SLAB], FP8, kind="ExternalInput").ap()
    xt_in = nc.dram_tensor("xt", [F_IN, SLAB], F32, kind="ExternalInput").ap()
    w1_in = nc.dram_tensor("w1", [F_IN, HID], F32, kind="ExternalInput").ap()
    b1_in = nc.dram_tensor("b1v", [HID], F32, kind="ExternalInput").ap()
    w2_in = nc.dram_tensor("w2", [HID, CLS], F32, kind="ExternalInput").ap()
    b2_in = nc.dram_tensor("b2v", [CLS], F32, kind="ExternalInput").ap()
    id_in = nc.dram_tensor("ident", [P, P], F32, kind="ExternalInput").ap()
    foldw_in = nc.dram_tensor("foldw", [P, HID], F32, kind="ExternalInput").ap()
    foldv_in = nc.dram_tensor("foldv", [104, CLS], F32, kind="ExternalInput").ap()
    out_d = nc.dram_tensor("out", [SLAB, CLS], F32, kind="ExternalOutput").ap()

    w_bounce = nc.dram_tensor("w_bounce", [SLAB, HID], F32)
    dsc_bounce = nc.dram_tensor("dsc_bounce", [SLAB], F32)
    dsc_full = nc.dram_tensor("dsc_full", [N], F32, addr_space="Shared")
    w_full = nc.dram_tensor("w_full", [N, HID], F32, addr_space="Shared")
    v_bounce = nc.dram_tensor("v_bounce", [SLAB, CLS], BF16)
    v_full = nc.dram_tensor("v_full", [N, CLS], BF16, addr_space="Shared")

    with tile.TileContext(nc) as tc, \
         tc.tile_pool(name="main", bufs=1) as mp, \
         tc.tile_pool(name="scratch", bufs=1) as scr, \
         tc.tile_pool(name="small2", bufs=2) as sp2, \
         tc.tile_pool(name="chain", bufs=1) as ch:

        # ---- constants / small loads -------------------------------------
        w1_sb = mp.tile([P, F_IN // P, HID], F32, name="w1_sb")
        nc.scalar.dma_start(out=w1_sb[:], in_=w1_in.rearrange("(c p) f -> p c f", p=P))
        w2_sb = mp.tile([HID, CLS], F32, name="w2_sb")
        nc.scalar.dma_start(out=w2_sb[:], in_=w2_in)
        b1c = mp.tile([HID, 1], F32, name="b1c")
        nc.scalar.dma_start(out=b1c[:], in_=b1_in.rearrange("(p o) -> p o", o=1))
        b2c = mp.tile([CLS, 1], F32, name="b2c")
        nc.scalar.dma_start(out=b2c[:], in_=b2_in.rearrange("(p o) -> p o", o=1))
        ident_sb = mp.tile([P, P], F32, name="ident_sb")
        nc.scalar.dma_start(out=ident_sb[:], in_=id_in)
        foldw_sb = mp.tile([P, HID], F32, name="foldw_sb")
        nc.scalar.dma_start(out=foldw_sb[:], in_=foldw_in)
        foldv_sb = mp.tile([104, CLS], F32, name="foldv_sb")
        nc.scalar.dma_start(out=foldv_sb[:], in_=foldv_in)
        ones_mat = mp.tile([P, P], FP8, name="ones_mat")
        nc.vector.memset(ones_mat[:], 1.0)

        # ---- resident adjacency slab: 12 tiles so deg can chase the load --
        DG = 8  # chunks per load DMA (1.57 MB each)
        b_tiles = []
        for d in range(JC // DG):
            bt = mp.tile([P, DG, SLAB], FP8, name=f"b_sb{d}", tag=f"b_sb{d}")
            b_tiles.append(bt)
            eng = nc.sync if d % 2 == 0 else nc.scalar
            eng.dma_start(
                out=bt[:],
                in_=b_in[d * DG * P:(d + 1) * DG * P, :]
                    .rearrange("(c p) f -> p c f", p=P))

        def b_slice(jc, sl):
            return b_tiles[jc // DG][:, jc % DG, sl]

        # ---- x @ W1 first (tiny; its DMAs must precede the big B load) ---
        wacc_a = mp.tile([P, LC, HID], F32, name="wacc_a")
        wacc_b = mp.tile([P, LC, HID], F32, name="wacc_b")
        accs = [wacc_a, wacc_b]
        with tc.tile_pool(name="ps_x", bufs=2, space="PSUM") as psxp:
            for kc in range(F_IN // P):
                xt_chunk = scr.tile([P, SLAB], F32, name="xt_chunk", tag="scratch")
                nc.scalar.dma_start(out=xt_chunk[:], in_=xt_in[kc * P:(kc + 1) * P, :])
                for l in range(LC):
                    psx = psxp.tile([P, HID], F32, name="psx", tag="psx")
                    nc.tensor.matmul(psx[:],
                                     xt_chunk[:, l * P:(l + 1) * P],
                                     w1_sb[:, kc, :], start=True, stop=True)
                    if kc == 0:
                        nc.vector.tensor_copy(wacc_a[:, l, :], psx[:])
                    else:
                        nc.vector.tensor_tensor(
                            out=accs[kc % 2][:, l, :],
                            in0=accs[(kc + 1) % 2][:, l, :], in1=psx[:],
                            op=mybir.AluOpType.add)
        xw1 = accs[(F_IN // P - 1) % 2]
        nc.scalar.dma_start(out=w_bounce.ap().rearrange("(c p) f -> p c f", p=P),
                            in_=xw1[:])
        nc.gpsimd.collective_compute(
            "AllGather", mybir.AluOpType.bypass,
            replica_groups=[list(range(R))],
            ins=[w_bounce[:]], outs=[w_full[:]])

        # ---- deg pass (single-stream, keeps PE warm under the load) ------
        with tc.tile_pool(name="ps_d", bufs=1, space="PSUM") as psd:
            psdeg = [psd.tile([P, 512], F32, name=f"psdeg{ib}", tag=f"psdeg{ib}")
                     for ib in range(NB)]
            for jc in range(JC):
                for ib in range(NB):
                    nc.tensor.matmul(
                        psdeg[ib][:],
                        ones_mat[:],
                        b_slice(jc, slice(ib * 512, (ib + 1) * 512)),
                        start=(jc == 0), stop=(jc == JC - 1))

            # ---- ds = sqrt(1 / (deg + 1)) --------------------------------
            dsum = mp.tile([1, SLAB], F32, name="dsum")
            for ib in range(NB):
                nc.vector.tensor_scalar_add(dsum[0:1, ib * 512:(ib + 1) * 512],
                                            psdeg[ib][0:1, :], 1.0)
            dinv = mp.tile([1, SLAB], F32, name="dinv")
            nc.vector.reciprocal(dinv[:], dsum[:])
            ds_row = mp.tile([1, SLAB], F32, name="ds_row")
            nc.scalar.activation(ds_row[:], dinv[:], mybir.ActivationFunctionType.Sqrt)

        ds_col = mp.tile([P, LC], F32, name="ds_col")
        with tc.tile_pool(name="ps_dt", bufs=2, space="PSUM") as psdt:
            for l in range(LC):
                pst_ds = psdt.tile([P, 1], F32, name="pst_ds", tag="pst_ds")
                nc.tensor.transpose(pst_ds[:], ds_row[0:1, l * P:(l + 1) * P],
                                    ident_sb[0:1, 0:1])
                nc.vector.tensor_copy(ds_col[:, l:l + 1], pst_ds[:])

        # ---- local w' chunks (for self-loop matmuls) ---------------------
        wprime = mp.tile([P, LC, HID], F32, name="wprime")
        for l in range(LC):
            nc.vector.tensor_scalar_mul(wprime[:, l, :], xw1[:, l, :],
                                        ds_col[:, l:l + 1])

        # ---- gather ds columns (tiny), then whl = bf16 split of ds * xW1 --
        nc.scalar.dma_start(out=dsc_bounce.ap().rearrange("(p c) -> p c", c=LC),
                            in_=ds_col[:])
        nc.gpsimd.collective_compute(
            "AllGather", mybir.AluOpType.bypass,
            replica_groups=[list(range(R))],
            ins=[dsc_bounce[:]], outs=[dsc_full[:]])
        ds_colf = mp.tile([P, R, LC, 1], F32, name="ds_colf")
        nc.scalar.dma_start(
            out=ds_colf[:],
            in_=dsc_full.ap().rearrange("(r p c o) -> p r c o", p=P, c=LC, o=1))

        wf_stage = scr.tile([P, R, LC, HID], F32, name="wf_stage", tag="scratch")
        nc.sync.dma_start(out=wf_stage[:],
                          in_=w_full.ap().rearrange("(r c p) f -> p r c f",
                                                    p=P, c=LC))
        wsc = scr.tile([P, R, LC, HID], F32, name="wsc", tag="scratch2")
        nc.vector.tensor_tensor(
            out=wsc[:], in0=wf_stage[:],
            in1=ds_colf[:].to_broadcast([P, R, LC, HID]),
            op=mybir.AluOpType.mult)
        whl = mp.tile([P, R, LC, 2 * HID], BF16, name="whl")  # [hi16 | lo16]
        nc.vector.tensor_copy(whl[:, :, :, 0:HID], wsc[:])
        nc.vector.tensor_tensor(out=whl[:, :, :, HID:2 * HID], in0=wsc[:],
                                in1=whl[:, :, :, 0:HID],
                                op=mybir.AluOpType.subtract)

        dsbc = mp.tile([CLS, SLAB], F32, name="dsbc")
        nc.gpsimd.partition_broadcast(dsbc[:], ds_row[:])

        # ---- pass 1 (4 col-strips of M=32) -------------------------------
        hpT = scr.tile([HID, SLAB], F32, name="hpT", tag="scratch2")
        with tc.tile_pool(name="ps_1", bufs=2, space="PSUM") as ps1p, \
             tc.tile_pool(name="ps_f", bufs=2, space="PSUM") as psfp:
            for ib in range(NB):
                sl = slice(ib * 512, (ib + 1) * 512)
                ps1 = ps1p.tile([P, 512], F32, name="ps1", tag="ps1")
                for jc in range(JC):
                    s = jc % 4
                    nc.tensor.matmul(ps1[32 * s:32 * s + 32, :],
                                     whl[:, jc // LC, jc % LC, :], b_slice(jc, sl),
                                     start=(jc < 4), stop=False,
                                     tile_position=(0, 32 * s),
                                     skip_group_check=True)
                for s in range(4):
                    nc.tensor.matmul(ps1[0:HID, s * P:(s + 1) * P],
                                     wprime[:, 4 * ib + s, :], ident_sb[:],
                                     start=False, stop=(s == 3),
                                     tile_position=(0, 0),
                                     skip_group_check=True)
                xfold = ch.tile([P, 512], F32, name="xfold", tag="xfold")
                nc.vector.tensor_copy(xfold[:], ps1[:])
                psagg = psfp.tile([HID, 512], F32, name="psagg", tag="psagg")
                nc.tensor.matmul(psagg[:], foldw_sb[:], xfold[:],
                                 start=True, stop=True)
                t1 = ch.tile([HID, 512], F32, name="t1", tag="c1")
                nc.vector.tensor_tensor(out=t1[:], in0=psagg[:],
                                        in1=dsbc[0:HID, sl],
                                        op=mybir.AluOpType.mult)
                h2 = ch.tile([HID, 512], F32, name="h2", tag="c2")
                nc.scalar.activation(h2[:], t1[:], mybir.ActivationFunctionType.Relu,
                                     bias=b1c[:, 0:1], scale=1.0)
                nc.vector.tensor_tensor(out=hpT[:, sl], in0=h2[:],
                                        in1=dsbc[0:HID, sl],
                                        op=mybir.AluOpType.mult)

        # ---- v = (ds*h) @ W2 (local f32 + bf16 for gather) ---------------
        vl_sb = mp.tile([P, LC, CLS], F32, name="vl_sb")
        vl_bf = mp.tile([P, LC, CLS], BF16, name="vl_bf")
        with tc.tile_pool(name="ps_v", bufs=2, space="PSUM") as psvp:
            for l in range(LC):
                psv = psvp.tile([P, CLS], F32, name="psv", tag="psv")
                nc.tensor.matmul(psv[:], hpT[:, l * P:(l + 1) * P], w2_sb[:],
                                 start=True, stop=True)
                nc.vector.tensor_copy(vl_sb[:, l, :], psv[:])
                nc.vector.tensor_copy(vl_bf[:, l, :], psv[:])
        nc.sync.dma_start(out=v_bounce.ap().rearrange("(c p) f -> p c f", p=P),
                          in_=vl_bf[:])
        nc.gpsimd.collective_compute(
            "AllGather", mybir.AluOpType.bypass,
            replica_groups=[list(range(R))],
            ins=[v_bounce[:]], outs=[v_full[:]])

        vhl = mp.tile([P, JC, CLS], BF16, name="vhl")
        nc.sync.dma_start(out=vhl[:],
                          in_=v_full.ap().rearrange("(c p) f -> p c f", p=P))

        # ---- pass 2 (2 col-strips of M=40) + log_softmax ------------------
        with tc.tile_pool(name="ps_2", bufs=2, space="PSUM") as ps2p, \
             tc.tile_pool(name="ps_g", bufs=2, space="PSUM") as psgp, \
             tc.tile_pool(name="ps_t", bufs=2, space="PSUM") as pstp:
            for ib in range(NB):
                sl = slice(ib * 512, (ib + 1) * 512)
                ps2 = ps2p.tile([104, 512], F32, name="ps2", tag="ps2")
                for jc in range(JC):
                    s = jc % 2
                    nc.tensor.matmul(ps2[64 * s:64 * s + CLS, :],
                                     vhl[:, jc, :], b_slice(jc, sl),
                                     start=(jc < 2), stop=False,
                                     tile_position=(0, 64 * s),
                                     skip_group_check=True)
                for s in range(4):
                    nc.tensor.matmul(ps2[0:CLS, s * P:(s + 1) * P],
                                     vl_sb[:, 4 * ib + s, :], ident_sb[:],
                                     start=False, stop=(s == 3),
                                     tile_position=(0, 0),
                                     skip_group_check=True)
                xfold2 = ch.tile([104, 512], F32, name="xfold2", tag="xfold")
                nc.vector.tensor_copy(xfold2[:], ps2[0:104, :])
                psagg2 = psgp.tile([CLS, 512], F32, name="psagg2", tag="psagg2")
                nc.tensor.matmul(psagg2[:], foldv_sb[:], xfold2[:],
                                 start=True, stop=True)
                lT = ch.tile([CLS, 512], F32, name="lT", tag="c1")
                nc.vector.tensor_tensor(out=lT[:], in0=psagg2[:],
                                        in1=dsbc[:, sl],
                                        op=mybir.AluOpType.mult)
                lT2 = ch.tile([CLS, 512], F32, name="lT2", tag="c2")
                nc.vector.tensor_scalar_add(lT2[:], lT[:], b2c[:, 0:1])

                for s in range(4):
                    pst = pstp.tile([P, CLS], F32, name="pst", tag="pst")
                    nc.tensor.transpose(pst[:], lT2[:, s * P:(s + 1) * P],
                                        ident_sb[0:CLS, 0:CLS])
                    nm = sp2.tile([P, 1], F32, name="nm", tag="nm")
                    nc.vector.tensor_reduce(nm[:], pst[:], mybir.AxisListType.X,
                                            mybir.AluOpType.max, negate=True)
                    e_sb = sp2.tile([P, CLS], F32, name="e_sb", tag="e_sb")
                    ssum = sp2.tile([P, 1], F32, name="ssum", tag="ssum")
                    nc.scalar.activation(e_sb[:], pst[:],
                                         mybir.ActivationFunctionType.Exp,
                                         bias=nm[:, 0:1], scale=1.0,
                                         accum_out=ssum[:, 0:1])
                    ls = sp2.tile([P, 1], F32, name="ls", tag="ls")
                    nc.scalar.activation(ls[:], ssum[:],
                                         mybir.ActivationFunctionType.Ln)
                    res = sp2.tile([P, CLS], F32, name="res", tag="res")
                    nc.vector.tensor_scalar(res[:], pst[:], nm[:, 0:1], ls[:, 0:1],
                                            op0=mybir.AluOpType.add,
                                            op1=mybir.AluOpType.subtract)
                    row = (ib * 4 + s) * P
                    nc.sync.dma_start(out=out_d[row:row + P, :], in_=res[:])

    nc.compile()
    return nc


def _prep_inputs(x, adj, W1, b1, W2, b2):
    x = np.ascontiguousarray(np.asarray(x, dtype=np.float32))
    adj = np.asarray(adj, dtype=np.float32)
    one8 = np.float32(1.0).astype(ml_dtypes.float8_e4m3).view(np.uint8)
    ident = np.eye(P, dtype=np.float32)
    pp, mm = np.arange(P)[:, None], np.arange(HID)[None, :]
    foldw = (pp % HID == mm).astype(np.float32)          # [128, 16]
    pp4, mm4 = np.arange(104)[:, None], np.arange(CLS)[None, :]
    foldv = ((pp4 == mm4) | (pp4 == mm4 + 64)).astype(np.float32)  # [104, 40]
    common = {
        "w1": np.ascontiguousarray(np.asarray(W1, np.float32)),
        "b1v": np.ascontiguousarray(np.asarray(b1, np.float32)),
        "w2": np.ascontiguousarray(np.asarray(W2, np.float32)),
        "b2v": np.ascontiguousarray(np.asarray(b2, np.float32)),
        "ident": ident, "foldw": foldw, "foldv": foldv,
    }
    in_maps = []
    for r in range(R):
        rows = slice(r * SLAB, (r + 1) * SLAB)
        bt = np.ascontiguousarray(adj[rows, :].T)          # [N, SLAB] f32
        b8 = np.where(bt != 0.0, one8, np.uint8(0)).view(ml_dtypes.float8_e4m3)
        xt = np.ascontiguousarray(x[rows, :].T)            # [F_IN, SLAB]
        in_maps.append({"b": b8, "xt": xt, **common})
    return in_maps


def _run(inputs, trace=False, **kw):
    if "nc" not in _CACHE:
        _CACHE["nc"] = _build()
    nc = _CACHE["nc"]
    in_maps = _prep_inputs(inputs["x"], inputs["adj"], inputs["W1"],
                           inputs["b1"], inputs["W2"], inputs["b2"])
    res = run_bass_kernel_spmd(nc, in_maps, core_ids=list(range(R)),
                               trace=trace, **kw)
    out = np.concatenate([res.results[r]["out"] for r in range(R)], axis=0)
    return out.astype(np.float32), res


def kernel(**inputs):
    out, _ = _run(inputs, trace=False)
    return out


# revision 19
# speedup vs baseline: 1.0637x; 1.0637x over previous
"""GCN (2-layer, gcn_norm) forward on 8 trn2 NeuronCores.

Math (reference):
    norm = D^-1/2 (A + I) D^-1/2
    h      = relu(norm @ (x @ W1) + b1)
    logits = norm @ (h @ W2) + b2
    out    = log_softmax(logits, axis=1)

Kernel strategy:
  - Never materialize norm:  norm @ v  =  ds * (A_hat @ (ds * v)),  ds = rsqrt(deg).
  - Host stages B = adj.T as per-core column slabs (row slab r of adj, transposed,
    C-contiguous) cast to fp8_e4m3 (0/1 entries are exact). TensorE contracts over
    the partition axis, so B's natural [j, i] layout makes A @ v a clean
    moving-operand matmul with perfect DMA patterns.
  - Each core holds its whole 18.9 MB fp8 slab RESIDENT in SBUF: HBM is read once.
  - 3 TensorE passes over the resident slab, all column-tiled so multiple
    chunk-streams run concurrently in the 128x128 PE array:
      deg    : 4 strips (M=1 ones-vector), hidden under the HBM load
      pass 1 : 4 strips of M=32 ([bf16-hi | bf16-lo] split features)
      pass 2 : 2 strips of M=40 (bf16 features)
    Strip partials are summed by one small fold matmul (f32) per block.
  - Self-loops (the +I) are added exactly via f32 identity-rhs matmuls into the
    same PSUM accumulation group, using the core-local f32 feature chunks.
  - Two tiny AllGathers: w' = ds*x@W1 (f32) and v = (ds*h)@W2 (bf16).
  - log_softmax per 128-row tile after a TensorE transpose back to [node, class].
"""
import numpy as np
import ml_dtypes

from concourse import bacc, mybir, tile
from concourse.bass_utils import run_bass_kernel_spmd

N = 12288
F_IN = 512
HID = 16
CLS = 40
R = 8                 # cores
SLAB = N // R         # 1536 rows per core
P = 128
JC = N // P           # 96 contraction chunks
LC = SLAB // P        # 12 local chunks
NB = SLAB // 512      # 3 i-blocks of 512
F32 = mybir.dt.float32
BF16 = mybir.dt.bfloat16
FP8 = mybir.dt.float8e4

_CACHE = {}


def _build():
    nc = bacc.Bacc("TRN2", target_bir_lowering=False, debug=False,
                   enable_asserts=True, num_devices=R)

    b_in = nc.dram_tensor("b", [N, SLAB], FP8, kind="ExternalInput").ap()
    xt_in = nc.dram_tensor("xt", [F_IN, SLAB], F32, kind="ExternalInput").ap()
    w1_in = nc.dram_tensor("w1", [F_IN, HID], F32, kind="ExternalInput").ap()
    b1_in = nc.dram_tensor("b1v", [HID], F32, kind="ExternalInput").ap()
    w2_in = nc.dram_tensor("w2", [HID, CLS], F32, kind="ExternalInput").ap()
    b2_in = nc.dram_tensor("b2v", [CLS], F32, kind="ExternalInput").ap()
    id_in = nc.dram_tensor("ident", [P, P], F32, kind="ExternalInput").ap()
    foldw_in = nc.dram_tensor("foldw", [P, HID], F32, kind="ExternalInput").ap()
    foldv_in = nc.dram_tensor("foldv", [104, CLS], F32, kind="ExternalInput").ap()
    out_d = nc.dram_tensor("out", [SLAB, CLS], F32, kind="ExternalOutput").ap()

    w_bounce = nc.dram_tensor("w_bounce", [SLAB, HID], F32)
    dsc_bounce = nc.dram_tensor("dsc_bounce", [SLAB], F32)
    dsc_full = nc.dram_tensor("dsc_full", [N], F32, addr_space="Shared")
    w_full = nc.dram_tensor("w_full", [N, HID], F32, addr_space="Shared")
    v_bounce = nc.dram_tensor("v_bounce", [SLAB, CLS], BF16)
    v_full = nc.dram_tensor("v_full", [N, CLS], BF16, addr_space="Shared")

    with tile.TileContext(nc) as tc, \
         tc.tile_pool(name="main", bufs=1) as mp, \
         tc.tile_pool(name="scratch", bufs=1) as scr, \
         tc.tile_pool(name="small2", bufs=2) as sp2, \
         tc.tile_pool(name="chain", bufs=1) as ch:

        # ---- constants / small loads -------------------------------------
        w1_sb = mp.tile([P, F_IN // P, HID], F32, name="w1_sb")
        nc.scalar.dma_start(out=w1_sb[:], in_=w1_in.rearrange("(c p) f -> p c f", p=P))
        w2_sb = mp.tile([HID, CLS], F32, name="w2_sb")
        nc.scalar.dma_start(out=w2_sb[:], in_=w2_in)
        b1c = mp.tile([HID, 1], F32, name="b1c")
        nc.scalar.dma_start(out=b1c[:], in_=b1_in.rearrange("(p o) -> p o", o=1))
        b2c = mp.tile([CLS, 1], F32, name="b2c")
        nc.scalar.dma_start(out=b2c[:], in_=b2_in.rearrange("(p o) -> p o", o=1))
        ident_sb = mp.tile([P, P], F32, name="ident_sb")
        nc.scalar.dma_start(out=ident_sb[:], in_=id_in)
        foldw_sb = mp.tile([P, HID], F32, name="foldw_sb")
        nc.scalar.dma_start(out=foldw_sb[:], in_=foldw_in)
        foldv_sb = mp.tile([104, CLS], F32, name="foldv_sb")
        nc.scalar.dma_start(out=foldv_sb[:], in_=foldv_in)
        ones_mat = mp.tile([P, P], FP8, name="ones_mat")
        nc.vector.memset(ones_mat[:], 1.0)

        # ---- x @ W1 first (tiny; its DMAs must precede the big B load) ---
        wacc_a = mp.tile([P, LC, HID], F32, name="wacc_a")
        wacc_b = mp.tile([P, LC, HID], F32, name="wacc_b")
        accs = [wacc_a, wacc_b]
        with tc.tile_pool(name="ps_x", bufs=2, space="PSUM") as psxp:
            for kc in range(F_IN // P):
                xt_chunk = scr.tile([P, SLAB], F32, name="xt_chunk", tag="scratch")
                nc.scalar.dma_start(out=xt_chunk[:], in_=xt_in[kc * P:(kc + 1) * P, :])
                for l in range(LC):
                    psx = psxp.tile([P, HID], F32, name="psx", tag="psx")
                    nc.tensor.matmul(psx[:],
                                     xt_chunk[:, l * P:(l + 1) * P],
                                     w1_sb[:, kc, :], start=True, stop=True)
                    if kc == 0:
                        nc.vector.tensor_copy(wacc_a[:, l, :], psx[:])
                    else:
                        nc.vector.tensor_tensor(
                            out=accs[kc % 2][:, l, :],
                            in0=accs[(kc + 1) % 2][:, l, :], in1=psx[:],
                            op=mybir.AluOpType.add)
        xw1 = accs[(F_IN // P - 1) % 2]
        nc.gpsimd.dma_start(out=w_bounce.ap().rearrange("(c p) f -> p c f", p=P),
                            in_=xw1[:])
        nc.gpsimd.collective_compute(
            "AllGather", mybir.AluOpType.bypass,
            replica_groups=[list(range(R))],
            ins=[w_bounce[:]], outs=[w_full[:]])

        # ---- resident adjacency slab: 12 tiles so deg can chase the load --
        DG = 8  # chunks per load DMA (1.57 MB each)
        b_tiles = []
        for d in range(JC // DG):
            bt = mp.tile([P, DG, SLAB], FP8, name=f"b_sb{d}", tag=f"b_sb{d}")
            b_tiles.append(bt)
            eng = nc.sync if d % 2 == 0 else nc.scalar
            eng.dma_start(
                out=bt[:],
                in_=b_in[d * DG * P:(d + 1) * DG * P, :]
                    .rearrange("(c p) f -> p c f", p=P))

        def b_slice(jc, sl):
            return b_tiles[jc // DG][:, jc % DG, sl]

        # ---- deg pass (single-stream, keeps PE warm under the load) ------
        with tc.tile_pool(name="ps_d", bufs=1, space="PSUM") as psd:
            psdeg = [psd.tile([P, 512], F32, name=f"psdeg{ib}", tag=f"psdeg{ib}")
                     for ib in range(NB)]
            for jc in range(JC):
                for ib in range(NB):
                    nc.tensor.matmul(
                        psdeg[ib][:],
                        ones_mat[:],
                        b_slice(jc, slice(ib * 512, (ib + 1) * 512)),
                        start=(jc == 0), stop=(jc == JC - 1))

            # ---- ds = sqrt(1 / (deg + 1)) --------------------------------
            dsum = mp.tile([1, SLAB], F32, name="dsum")
            for ib in range(NB):
                nc.vector.tensor_scalar_add(dsum[0:1, ib * 512:(ib + 1) * 512],
                                            psdeg[ib][0:1, :], 1.0)
            dinv = mp.tile([1, SLAB], F32, name="dinv")
            nc.vector.reciprocal(dinv[:], dsum[:])
            ds_row = mp.tile([1, SLAB], F32, name="ds_row")
            nc.scalar.activation(ds_row[:], dinv[:], mybir.ActivationFunctionType.Sqrt)

        ds_col = mp.tile([P, LC], F32, name="ds_col")
        with tc.tile_pool(name="ps_dt", bufs=2, space="PSUM") as psdt:
            for l in range(LC):
                pst_ds = psdt.tile([P, 1], F32, name="pst_ds", tag="pst_ds")
                nc.tensor.transpose(pst_ds[:], ds_row[0:1, l * P:(l + 1) * P],
                                    ident_sb[0:1, 0:1])
                nc.vector.tensor_copy(ds_col[:, l:l + 1], pst_ds[:])

        # ---- local w' chunks (for self-loop matmuls) ---------------------
        wprime = mp.tile([P, LC, HID], F32, name="wprime")
        for l in range(LC):
            nc.vector.tensor_scalar_mul(wprime[:, l, :], xw1[:, l, :],
                                        ds_col[:, l:l + 1])

        # ---- gather ds columns (tiny), then whl = bf16 split of ds * xW1 --
        nc.gpsimd.dma_start(out=dsc_bounce.ap().rearrange("(p c) -> p c", c=LC),
                            in_=ds_col[:])
        nc.gpsimd.collective_compute(
            "AllGather", mybir.AluOpType.bypass,
            replica_groups=[list(range(R))],
            ins=[dsc_bounce[:]], outs=[dsc_full[:]])
        ds_colf = mp.tile([P, R, LC, 1], F32, name="ds_colf")
        nc.gpsimd.dma_start(
            out=ds_colf[:],
            in_=dsc_full.ap().rearrange("(r p c o) -> p r c o", p=P, c=LC, o=1))

        wf_stage = scr.tile([P, R, LC, HID], F32, name="wf_stage", tag="scratch")
        nc.gpsimd.dma_start(out=wf_stage[:],
                          in_=w_full.ap().rearrange("(r c p) f -> p r c f",
                                                    p=P, c=LC))
        wsc = scr.tile([P, R, LC, HID], F32, name="wsc", tag="scratch2")
        nc.vector.tensor_tensor(
            out=wsc[:], in0=wf_stage[:],
            in1=ds_colf[:].to_broadcast([P, R, LC, HID]),
            op=mybir.AluOpType.mult)
        whl = mp.tile([P, R, LC, 2 * HID], BF16, name="whl")  # [hi16 | lo16]
        nc.vector.tensor_copy(whl[:, :, :, 0:HID], wsc[:])
        nc.vector.tensor_tensor(out=whl[:, :, :, HID:2 * HID], in0=wsc[:],
                                in1=whl[:, :, :, 0:HID],
                                op=mybir.AluOpType.subtract)

        dsbc = mp.tile([CLS, SLAB], F32, name="dsbc")
        nc.gpsimd.partition_broadcast(dsbc[:], ds_row[:])

        # ---- pass 1 (4 col-strips of M=32; 3 blocks in flight) -----------
        hpT = scr.tile([HID, SLAB], F32, name="hpT", tag="scratch2")
        with tc.tile_pool(name="ps_1", bufs=1, space="PSUM") as ps1p, \
             tc.tile_pool(name="ps_f", bufs=2, space="PSUM") as psfp:
            ps1s = []
            for ib in range(NB):
                sl = slice(ib * 512, (ib + 1) * 512)
                ps1 = ps1p.tile([P, 512], F32, name=f"ps1_{ib}", tag=f"ps1_{ib}")
                ps1s.append(ps1)
                for jc in range(JC):
                    s = jc % 4
                    nc.tensor.matmul(ps1[32 * s:32 * s + 32, :],
                                     whl[:, jc // LC, jc % LC, :], b_slice(jc, sl),
                                     start=(jc < 4), stop=False,
                                     tile_position=(0, 32 * s),
                                     skip_group_check=True)
                for s in range(4):
                    nc.tensor.matmul(ps1[0:HID, s * P:(s + 1) * P],
                                     wprime[:, 4 * ib + s, :], ident_sb[:],
                                     start=False, stop=(s == 3),
                                     tile_position=(0, 0),
                                     skip_group_check=True)
            for ib in range(NB):
                sl = slice(ib * 512, (ib + 1) * 512)
                xfold = ch.tile([P, 512], F32, name="xfold", tag="xfold")
                nc.vector.tensor_copy(xfold[:], ps1s[ib][:])
                psagg = psfp.tile([HID, 512], F32, name="psagg", tag="psagg")
                nc.tensor.matmul(psagg[:], foldw_sb[:], xfold[:],
                                 start=True, stop=True)
                t1 = ch.tile([HID, 512], F32, name="t1", tag="c1")
                nc.vector.tensor_tensor(out=t1[:], in0=psagg[:],
                                        in1=dsbc[0:HID, sl],
                                        op=mybir.AluOpType.mult)
                h2 = ch.tile([HID, 512], F32, name="h2", tag="c2")
                nc.scalar.activation(h2[:], t1[:], mybir.ActivationFunctionType.Relu,
                                     bias=b1c[:, 0:1], scale=1.0)
                nc.vector.tensor_tensor(out=hpT[:, sl], in0=h2[:],
                                        in1=dsbc[0:HID, sl],
                                        op=mybir.AluOpType.mult)

        # ---- v = (ds*h) @ W2 (local f32 + bf16 for gather) ---------------
        vl_sb = mp.tile([P, LC, CLS], F32, name="vl_sb")
        vl_bf = mp.tile([P, LC, CLS], BF16, name="vl_bf")
        with tc.tile_pool(name="ps_v", bufs=2, space="PSUM") as psvp:
            for l in range(LC):
                psv = psvp.tile([P, CLS], F32, name="psv", tag="psv")
                nc.tensor.matmul(psv[:], hpT[:, l * P:(l + 1) * P], w2_sb[:],
                                 start=True, stop=True)
                nc.vector.tensor_copy(vl_sb[:, l, :], psv[:])
                nc.vector.tensor_copy(vl_bf[:, l, :], psv[:])
        nc.gpsimd.dma_start(out=v_bounce.ap().rearrange("(c p) f -> p c f", p=P),
                          in_=vl_bf[:])
        nc.gpsimd.collective_compute(
            "AllGather", mybir.AluOpType.bypass,
            replica_groups=[list(range(R))],
            ins=[v_bounce[:]], outs=[v_full[:]])

        vhl = mp.tile([P, JC, CLS], BF16, name="vhl")
        nc.gpsimd.dma_start(out=vhl[:],
                          in_=v_full.ap().rearrange("(c p) f -> p c f", p=P))

        # ---- pass 2 (2 col-strips of M=40; 3 blocks in flight) ------------
        with tc.tile_pool(name="ps_2", bufs=1, space="PSUM") as ps2p, \
             tc.tile_pool(name="ps_g", bufs=2, space="PSUM") as psgp, \
             tc.tile_pool(name="ps_t", bufs=2, space="PSUM") as pstp:
            ps2s = []
            for ib in range(NB):
                sl = slice(ib * 512, (ib + 1) * 512)
                ps2 = ps2p.tile([104, 512], F32, name=f"ps2_{ib}", tag=f"ps2_{ib}")
                ps2s.append(ps2)
                for jc in range(JC):
                    s = jc % 2
                    nc.tensor.matmul(ps2[64 * s:64 * s + CLS, :],
                                     vhl[:, jc, :], b_slice(jc, sl),
                                     start=(jc < 2), stop=False,
                                     tile_position=(0, 64 * s),
                                     skip_group_check=True)
                for s in range(4):
                    nc.tensor.matmul(ps2[0:CLS, s * P:(s + 1) * P],
                                     vl_sb[:, 4 * ib + s, :], ident_sb[:],
                                     start=False, stop=(s == 3),
                                     tile_position=(0, 0),
                                     skip_group_check=True)
            for ib in range(NB):
                sl = slice(ib * 512, (ib + 1) * 512)
                xfold2 = ch.tile([104, 512], F32, name="xfold2", tag="xfold")
                nc.vector.tensor_copy(xfold2[:], ps2s[ib][0:104, :])
                psagg2 = psgp.tile([CLS, 512], F32, name="psagg2", tag="psagg2")
                nc.tensor.matmul(psagg2[:], foldv_sb[:], xfold2[:],
                                 start=True, stop=True)
                lT = ch.tile([CLS, 512], F32, name="lT", tag="c1")
                nc.vector.tensor_tensor(out=lT[:], in0=psagg2[:],
                                        in1=dsbc[:, sl],
                                        op=mybir.AluOpType.mult)
                lT2 = ch.tile([CLS, 512], F32, name="lT2", tag="c2")
                nc.vector.tensor_scalar_add(lT2[:], lT[:], b2c[:, 0:1])

                for s in range(4):
                    pst = pstp.tile([P, CLS], F32, name="pst", tag="pst")
                    nc.tensor.transpose(pst[:], lT2[:, s * P:(s + 1) * P],
                                        ident_sb[0:CLS, 0:CLS])
                    nm = sp2.tile([P, 1], F32, name="nm", tag="nm")
                    nc.vector.tensor_reduce(nm[:], pst[:], mybir.AxisListType.X,
                                            mybir.AluOpType.max, negate=True)
                    e_sb = sp2.tile([P, CLS], F32, name="e_sb", tag="e_sb")
                    ssum = sp2.tile([P, 1], F32, name="ssum", tag="ssum")
                    nc.scalar.activation(e_sb[:], pst[:],
                                         mybir.ActivationFunctionType.Exp,
                                         bias=nm[:, 0:1], scale=1.0,
                                         accum_out=ssum[:, 0:1])
                    ls = sp2.tile([P, 1], F32, name="ls", tag="ls")
                    nc.scalar.activation(ls[:], ssum[:],
                                         mybir.ActivationFunctionType.Ln)
                    res = sp2.tile([P, CLS], F32, name="res", tag="res")
                    nc.vector.tensor_scalar(res[:], pst[:], nm[:, 0:1], ls[:, 0:1],
                                            op0=mybir.AluOpType.add,
                                            op1=mybir.AluOpType.subtract)
                    row = (ib * 4 + s) * P
                    nc.sync.dma_start(out=out_d[row:row + P, :], in_=res[:])

    nc.compile()
    return nc


def _prep_inputs(x, adj, W1, b1, W2, b2):
    x = np.ascontiguousarray(np.asarray(x, dtype=np.float32))
    adj = np.asarray(adj, dtype=np.float32)
    one8 = np.float32(1.0).astype(ml_dtypes.float8_e4m3).view(np.uint8)
    ident = np.eye(P, dtype=np.float32)
    pp, mm = np.arange(P)[:, None], np.arange(HID)[None, :]
    foldw = (pp % HID == mm).astype(np.float32)          # [128, 16]
    pp4, mm4 = np.arange(104)[:, None], np.arange(CLS)[None, :]
    foldv = ((pp4 == mm4) | (pp4 == mm4 + 64)).astype(np.float32)  # [104, 40]
    common = {
        "w1": np.ascontiguousarray(np.asarray(W1, np.float32)),
        "b1v": np.ascontiguousarray(np.asarray(b1, np.float32)),
        "w2": np.ascontiguousarray(np.asarray(W2, np.float32)),
        "b2v": np.ascontiguousarray(np.asarray(b2, np.float32)),
        "ident": ident, "foldw": foldw, "foldv": foldv,
    }
    in_maps = []
    for r in range(R):
        rows = slice(r * SLAB, (r + 1) * SLAB)
        bt = np.ascontiguousarray(adj[rows, :].T)          # [N, SLAB] f32
        b8 = np.where(bt != 0.0, one8, np.uint8(0)).view(ml_dtypes.float8_e4m3)
        xt = np.ascontiguousarray(x[rows, :].T)            # [F_IN, SLAB]
        in_maps.append({"b": b8, "xt": xt, **common})
    return in_maps


def _run(inputs, trace=False, **kw):
    if "nc" not in _CACHE:
        _CACHE["nc"] = _build()
    nc = _CACHE["nc"]
    in_maps = _prep_inputs(inputs["x"], inputs["adj"], inputs["W1"],
                           inputs["b1"], inputs["W2"], inputs["b2"])
    res = run_bass_kernel_spmd(nc, in_maps, core_ids=list(range(R)),
                               trace=trace, **kw)
    out = np.concatenate([res.results[r]["out"] for r in range(R)], axis=0)
    return out.astype(np.float32), res


def kernel(**inputs):
    out, _ = _run(inputs, trace=False)
    return out


# revision 21
# speedup vs baseline: 1.0889x; 1.0236x over previous
"""GCN (2-layer, gcn_norm) forward on 8 trn2 NeuronCores.

Math (reference):
    norm = D^-1/2 (A + I) D^-1/2
    h      = relu(norm @ (x @ W1) + b1)
    logits = norm @ (h @ W2) + b2
    out    = log_softmax(logits, axis=1)

Kernel strategy:
  - Never materialize norm:  norm @ v  =  ds * (A_hat @ (ds * v)),  ds = rsqrt(deg).
  - Host stages B = adj.T as per-core column slabs (row slab r of adj, transposed,
    C-contiguous) cast to fp8_e4m3 (0/1 entries are exact). TensorE contracts over
    the partition axis, so B's natural [j, i] layout makes A @ v a clean
    moving-operand matmul with perfect DMA patterns.
  - Each core holds its whole 18.9 MB fp8 slab RESIDENT in SBUF: HBM is read once.
  - 3 TensorE passes over the resident slab, all column-tiled so multiple
    chunk-streams run concurrently in the 128x128 PE array:
      deg    : 4 strips (M=1 ones-vector), hidden under the HBM load
      pass 1 : 4 strips of M=32 ([bf16-hi | bf16-lo] split features)
      pass 2 : 2 strips of M=40 (bf16 features)
    Strip partials are summed by one small fold matmul (f32) per block.
  - Self-loops (the +I) are added exactly via f32 identity-rhs matmuls into the
    same PSUM accumulation group, using the core-local f32 feature chunks.
  - Two tiny AllGathers: w' = ds*x@W1 (f32) and v = (ds*h)@W2 (bf16).
  - log_softmax per 128-row tile after a TensorE transpose back to [node, class].
"""
import numpy as np
import ml_dtypes

from concourse import bacc, mybir, tile
from concourse.bass_utils import run_bass_kernel_spmd

N = 12288
F_IN = 512
HID = 16
CLS = 40
R = 8                 # cores
SLAB = N // R         # 1536 rows per core
P = 128
JC = N // P           # 96 contraction chunks
LC = SLAB // P        # 12 local chunks
NB = SLAB // 512      # 3 i-blocks of 512
F32 = mybir.dt.float32
BF16 = mybir.dt.bfloat16
FP8 = mybir.dt.float8e4

_CACHE = {}


def _build():
    nc = bacc.Bacc("TRN2", target_bir_lowering=False, debug=False,
                   enable_asserts=True, num_devices=R)

    b_in = nc.dram_tensor("b", [N, SLAB], FP8, kind="ExternalInput").ap()
    xt_in = nc.dram_tensor("xt", [F_IN, SLAB], F32, kind="ExternalInput").ap()
    w1_in = nc.dram_tensor("w1", [F_IN, HID], F32, kind="ExternalInput").ap()
    b1_in = nc.dram_tensor("b1v", [HID], F32, kind="ExternalInput").ap()
    w2_in = nc.dram_tensor("w2", [HID, CLS], F32, kind="ExternalInput").ap()
    b2_in = nc.dram_tensor("b2v", [CLS], F32, kind="ExternalInput").ap()
    id_in = nc.dram_tensor("ident", [P, P], F32, kind="ExternalInput").ap()
    foldw_in = nc.dram_tensor("foldw", [P, HID], F32, kind="ExternalInput").ap()
    foldv_in = nc.dram_tensor("foldv", [104, CLS], F32, kind="ExternalInput").ap()
    out_d = nc.dram_tensor("out", [SLAB, CLS], F32, kind="ExternalOutput").ap()

    w_bounce = nc.dram_tensor("w_bounce", [SLAB, HID], F32)
    dsc_bounce = nc.dram_tensor("dsc_bounce", [SLAB], F32)
    dsc_full = nc.dram_tensor("dsc_full", [N], F32, addr_space="Shared")
    w_full = nc.dram_tensor("w_full", [N, HID], F32, addr_space="Shared")
    v_bounce = nc.dram_tensor("v_bounce", [SLAB, CLS], BF16)
    v_full = nc.dram_tensor("v_full", [N, CLS], BF16, addr_space="Shared")

    with tile.TileContext(nc) as tc, \
         tc.tile_pool(name="main", bufs=1) as mp, \
         tc.tile_pool(name="scratch", bufs=1) as scr, \
         tc.tile_pool(name="small2", bufs=2) as sp2, \
         tc.tile_pool(name="chain", bufs=1) as ch:

        # ---- constants / small loads -------------------------------------
        w1_sb = mp.tile([P, F_IN // P, HID], F32, name="w1_sb")
        nc.scalar.dma_start(out=w1_sb[:], in_=w1_in.rearrange("(c p) f -> p c f", p=P))
        w2_sb = mp.tile([HID, CLS], F32, name="w2_sb")
        nc.scalar.dma_start(out=w2_sb[:], in_=w2_in)
        b1c = mp.tile([HID, 1], F32, name="b1c")
        nc.scalar.dma_start(out=b1c[:], in_=b1_in.rearrange("(p o) -> p o", o=1))
        b2c = mp.tile([CLS, 1], F32, name="b2c")
        nc.scalar.dma_start(out=b2c[:], in_=b2_in.rearrange("(p o) -> p o", o=1))
        ident_sb = mp.tile([P, P], F32, name="ident_sb")
        nc.scalar.dma_start(out=ident_sb[:], in_=id_in)
        foldw_sb = mp.tile([P, HID], F32, name="foldw_sb")
        nc.scalar.dma_start(out=foldw_sb[:], in_=foldw_in)
        foldv_sb = mp.tile([104, CLS], F32, name="foldv_sb")
        nc.scalar.dma_start(out=foldv_sb[:], in_=foldv_in)
        ones_mat = mp.tile([P, P], FP8, name="ones_mat")
        nc.vector.memset(ones_mat[:], 1.0)

        # ---- x @ W1 first (PSUM-grouped accumulate over the 4 K-chunks) --
        xw1_sb = mp.tile([P, LC, HID], F32, name="xw1_sb")
        with tc.tile_pool(name="ps_x", bufs=1, space="PSUM") as psxp:
            psxg = [psxp.tile([P, 4, HID], F32, name=f"psxg{g}", tag=f"psxg{g}")
                    for g in range(3)]
            for kc in range(F_IN // P):
                xt_chunk = scr.tile([P, SLAB], F32, name="xt_chunk", tag="scratch")
                nc.gpsimd.dma_start(out=xt_chunk[:], in_=xt_in[kc * P:(kc + 1) * P, :])
                for l in range(LC):
                    nc.tensor.matmul(psxg[l // 4][:, l % 4, :],
                                     xt_chunk[:, l * P:(l + 1) * P],
                                     w1_sb[:, kc, :],
                                     start=(kc == 0 and l % 4 == 0),
                                     stop=(kc == F_IN // P - 1),
                                     skip_group_check=True)
            for g in range(3):
                nc.vector.tensor_copy(xw1_sb[:, 4 * g:4 * g + 4, :], psxg[g][:])
        xw1 = xw1_sb
        nc.gpsimd.dma_start(out=w_bounce.ap().rearrange("(c p) f -> p c f", p=P),
                            in_=xw1[:])
        nc.gpsimd.collective_compute(
            "AllGather", mybir.AluOpType.bypass,
            replica_groups=[list(range(R))],
            ins=[w_bounce[:]], outs=[w_full[:]])
        # ---- resident adjacency slab: 12 tiles so deg can chase the load --
        DG = 8  # chunks per load DMA (1.57 MB each)
        b_tiles = []
        for d in range(JC // DG):
            bt = mp.tile([P, DG, SLAB], FP8, name=f"b_sb{d}", tag=f"b_sb{d}")
            b_tiles.append(bt)
            eng = nc.sync if d % 2 == 0 else nc.scalar
            eng.dma_start(
                out=bt[:],
                in_=b_in[d * DG * P:(d + 1) * DG * P, :]
                    .rearrange("(c p) f -> p c f", p=P))

        def b_slice(jc, sl):
            return b_tiles[jc // DG][:, jc % DG, sl]

        # ---- deg pass (single-stream, keeps PE warm under the load) ------
        with tc.tile_pool(name="ps_d", bufs=1, space="PSUM") as psd:
            psdeg = [psd.tile([P, 512], F32, name=f"psdeg{ib}", tag=f"psdeg{ib}")
                     for ib in range(NB)]
            for jc in range(JC):
                for ib in range(NB):
                    nc.tensor.matmul(
                        psdeg[ib][:],
                        ones_mat[:],
                        b_slice(jc, slice(ib * 512, (ib + 1) * 512)),
                        start=(jc == 0), stop=(jc == JC - 1))

            # ---- ds = sqrt(1 / (deg + 1)) --------------------------------
            dsum = mp.tile([1, SLAB], F32, name="dsum")
            for ib in range(NB):
                nc.vector.tensor_scalar_add(dsum[0:1, ib * 512:(ib + 1) * 512],
                                            psdeg[ib][0:1, :], 1.0)
            dinv = mp.tile([1, SLAB], F32, name="dinv")
            nc.vector.reciprocal(dinv[:], dsum[:])
            ds_row = mp.tile([1, SLAB], F32, name="ds_row")
            nc.scalar.activation(ds_row[:], dinv[:], mybir.ActivationFunctionType.Sqrt)

        ds_col = mp.tile([P, LC], F32, name="ds_col")
        with tc.tile_pool(name="ps_dt", bufs=2, space="PSUM") as psdt:
            for l in range(LC):
                pst_ds = psdt.tile([P, 1], F32, name="pst_ds", tag="pst_ds")
                nc.tensor.transpose(pst_ds[:], ds_row[0:1, l * P:(l + 1) * P],
                                    ident_sb[0:1, 0:1])
                nc.vector.tensor_copy(ds_col[:, l:l + 1], pst_ds[:])

        # ---- local w' chunks (for self-loop matmuls) ---------------------
        wprime = mp.tile([P, LC, HID], F32, name="wprime")
        for l in range(LC):
            nc.vector.tensor_scalar_mul(wprime[:, l, :], xw1[:, l, :],
                                        ds_col[:, l:l + 1])

        # ---- gather ds columns (tiny), then whl = bf16 split of ds * xW1 --
        nc.gpsimd.dma_start(out=dsc_bounce.ap().rearrange("(p c) -> p c", c=LC),
                            in_=ds_col[:])
        nc.gpsimd.collective_compute(
            "AllGather", mybir.AluOpType.bypass,
            replica_groups=[list(range(R))],
            ins=[dsc_bounce[:]], outs=[dsc_full[:]])
        ds_colf = mp.tile([P, R, LC, 1], F32, name="ds_colf")
        nc.gpsimd.dma_start(
            out=ds_colf[:],
            in_=dsc_full.ap().rearrange("(r p c o) -> p r c o", p=P, c=LC, o=1))

        wf_stage = scr.tile([P, R, LC, HID], F32, name="wf_stage", tag="scratch")
        nc.gpsimd.dma_start(out=wf_stage[:],
                          in_=w_full.ap().rearrange("(r c p) f -> p r c f",
                                                    p=P, c=LC))
        wsc = scr.tile([P, R, LC, HID], F32, name="wsc", tag="scratch2")
        nc.vector.tensor_tensor(
            out=wsc[:], in0=wf_stage[:],
            in1=ds_colf[:].to_broadcast([P, R, LC, HID]),
            op=mybir.AluOpType.mult)
        whl = mp.tile([P, R, LC, 2 * HID], BF16, name="whl")  # [hi16 | lo16]
        nc.vector.tensor_copy(whl[:, :, :, 0:HID], wsc[:])
        nc.vector.tensor_tensor(out=whl[:, :, :, HID:2 * HID], in0=wsc[:],
                                in1=whl[:, :, :, 0:HID],
                                op=mybir.AluOpType.subtract)

        dsbc = mp.tile([CLS, SLAB], F32, name="dsbc")
        nc.gpsimd.partition_broadcast(dsbc[:], ds_row[:])

        # ---- pass 1 (4 col-strips of M=32; 3 blocks in flight) -----------
        hpT = scr.tile([HID, SLAB], F32, name="hpT", tag="scratch2")
        with tc.tile_pool(name="ps_1", bufs=1, space="PSUM") as ps1p, \
             tc.tile_pool(name="ps_f", bufs=2, space="PSUM") as psfp:
            ps1s = []
            for ib in range(NB):
                sl = slice(ib * 512, (ib + 1) * 512)
                ps1 = ps1p.tile([P, 512], F32, name=f"ps1_{ib}", tag=f"ps1_{ib}")
                ps1s.append(ps1)
                for jc in range(JC):
                    s = jc % 4
                    nc.tensor.matmul(ps1[32 * s:32 * s + 32, :],
                                     whl[:, jc // LC, jc % LC, :], b_slice(jc, sl),
                                     start=(jc < 4), stop=False,
                                     tile_position=(0, 32 * s),
                                     skip_group_check=True)
                for s in range(4):
                    nc.tensor.matmul(ps1[0:HID, s * P:(s + 1) * P],
                                     wprime[:, 4 * ib + s, :], ident_sb[:],
                                     start=False, stop=(s == 3),
                                     tile_position=(0, 0),
                                     skip_group_check=True)
            for ib in range(NB):
                sl = slice(ib * 512, (ib + 1) * 512)
                xfold = ch.tile([P, 512], F32, name="xfold", tag="xfold")
                nc.vector.tensor_copy(xfold[:], ps1s[ib][:])
                psagg = psfp.tile([HID, 512], F32, name="psagg", tag="psagg")
                nc.tensor.matmul(psagg[:], foldw_sb[:], xfold[:],
                                 start=True, stop=True)
                t1 = ch.tile([HID, 512], F32, name="t1", tag="c1")
                nc.vector.tensor_tensor(out=t1[:], in0=psagg[:],
                                        in1=dsbc[0:HID, sl],
                                        op=mybir.AluOpType.mult)
                h2 = ch.tile([HID, 512], F32, name="h2", tag="c2")
                nc.scalar.activation(h2[:], t1[:], mybir.ActivationFunctionType.Relu,
                                     bias=b1c[:, 0:1], scale=1.0)
                nc.vector.tensor_tensor(out=hpT[:, sl], in0=h2[:],
                                        in1=dsbc[0:HID, sl],
                                        op=mybir.AluOpType.mult)

        # ---- v = (ds*h) @ W2 (local f32 + bf16 for gather) ---------------
        vl_sb = mp.tile([P, LC, CLS], F32, name="vl_sb")
        vl_bf = mp.tile([P, LC, CLS], BF16, name="vl_bf")
        with tc.tile_pool(name="ps_v", bufs=2, space="PSUM") as psvp:
            for l in range(LC):
                psv = psvp.tile([P, CLS], F32, name="psv", tag="psv")
                nc.tensor.matmul(psv[:], hpT[:, l * P:(l + 1) * P], w2_sb[:],
                                 start=True, stop=True)
                nc.vector.tensor_copy(vl_sb[:, l, :], psv[:])
                nc.vector.tensor_copy(vl_bf[:, l, :], psv[:])
        nc.gpsimd.dma_start(out=v_bounce.ap().rearrange("(c p) f -> p c f", p=P),
                          in_=vl_bf[:])
        nc.gpsimd.collective_compute(
            "AllGather", mybir.AluOpType.bypass,
            replica_groups=[list(range(R))],
            ins=[v_bounce[:]], outs=[v_full[:]])

        vhl = mp.tile([P, JC, CLS], BF16, name="vhl")
        nc.gpsimd.dma_start(out=vhl[:],
                          in_=v_full.ap().rearrange("(c p) f -> p c f", p=P))

        # ---- pass 2 (2 col-strips of M=40; 3 blocks in flight) ------------
        with tc.tile_pool(name="ps_2", bufs=1, space="PSUM") as ps2p, \
             tc.tile_pool(name="ps_g", bufs=2, space="PSUM") as psgp, \
             tc.tile_pool(name="ps_t", bufs=2, space="PSUM") as pstp:
            ps2s = []
            for ib in range(NB):
                sl = slice(ib * 512, (ib + 1) * 512)
                ps2 = ps2p.tile([104, 512], F32, name=f"ps2_{ib}", tag=f"ps2_{ib}")
                ps2s.append(ps2)
                for jc in range(JC):
                    s = jc % 2
                    nc.tensor.matmul(ps2[64 * s:64 * s + CLS, :],
                                     vhl[:, jc, :], b_slice(jc, sl),
                                     start=(jc < 2), stop=False,
                                     tile_position=(0, 64 * s),
                                     skip_group_check=True)
                for s in range(4):
                    nc.tensor.matmul(ps2[0:CLS, s * P:(s + 1) * P],
                                     vl_sb[:, 4 * ib + s, :], ident_sb[:],
                                     start=False, stop=(s == 3),
                                     tile_position=(0, 0),
                                     skip_group_check=True)
            for ib in range(NB):
                sl = slice(ib * 512, (ib + 1) * 512)
                xfold2 = ch.tile([104, 512], F32, name="xfold2", tag="xfold")
                nc.vector.tensor_copy(xfold2[:], ps2s[ib][0:104, :])
                psagg2 = psgp.tile([CLS, 512], F32, name="psagg2", tag="psagg2")
                nc.tensor.matmul(psagg2[:], foldv_sb[:], xfold2[:],
                                 start=True, stop=True)
                lT = ch.tile([CLS, 512], F32, name="lT", tag="c1")
                nc.vector.tensor_tensor(out=lT[:], in0=psagg2[:],
                                        in1=dsbc[:, sl],
                                        op=mybir.AluOpType.mult)
                lT2 = ch.tile([CLS, 512], F32, name="lT2", tag="c2")
                nc.vector.tensor_scalar_add(lT2[:], lT[:], b2c[:, 0:1])

                for s in range(4):
                    pst = pstp.tile([P, CLS], F32, name="pst", tag="pst")
                    nc.tensor.transpose(pst[:], lT2[:, s * P:(s + 1) * P],
                                        ident_sb[0:CLS, 0:CLS])
                    nm = sp2.tile([P, 1], F32, name="nm", tag="nm")
                    nc.vector.tensor_reduce(nm[:], pst[:], mybir.AxisListType.X,
                                            mybir.AluOpType.max, negate=True)
                    e_sb = sp2.tile([P, CLS], F32, name="e_sb", tag="e_sb")
                    ssum = sp2.tile([P, 1], F32, name="ssum", tag="ssum")
                    nc.scalar.activation(e_sb[:], pst[:],
                                         mybir.ActivationFunctionType.Exp,
                                         bias=nm[:, 0:1], scale=1.0,
                                         accum_out=ssum[:, 0:1])
                    ls = sp2.tile([P, 1], F32, name="ls", tag="ls")
                    nc.scalar.activation(ls[:], ssum[:],
                                         mybir.ActivationFunctionType.Ln)
                    res = sp2.tile([P, CLS], F32, name="res", tag="res")
                    nc.vector.tensor_scalar(res[:], pst[:], nm[:, 0:1], ls[:, 0:1],
                                            op0=mybir.AluOpType.add,
                                            op1=mybir.AluOpType.subtract)
                    row = (ib * 4 + s) * P
                    nc.sync.dma_start(out=out_d[row:row + P, :], in_=res[:])

    nc.compile()
    return nc


def _prep_inputs(x, adj, W1, b1, W2, b2):
    x = np.ascontiguousarray(np.asarray(x, dtype=np.float32))
    adj = np.asarray(adj, dtype=np.float32)
    one8 = np.float32(1.0).astype(ml_dtypes.float8_e4m3).view(np.uint8)
    ident = np.eye(P, dtype=np.float32)
    pp, mm = np.arange(P)[:, None], np.arange(HID)[None, :]
    foldw = (pp % HID == mm).astype(np.float32)          # [128, 16]
    pp4, mm4 = np.arange(104)[:, None], np.arange(CLS)[None, :]
    foldv = ((pp4 == mm4) | (pp4 == mm4 + 64)).astype(np.float32)  # [104, 40]
    common = {
        "w1": np.ascontiguousarray(np.asarray(W1, np.float32)),
        "b1v": np.ascontiguousarray(np.asarray(b1, np.float32)),
        "w2": np.ascontiguousarray(np.asarray(W2, np.float32)),
        "b2v": np.ascontiguousarray(np.asarray(b2, np.float32)),
        "ident": ident, "foldw": foldw, "foldv": foldv,
    }
    in_maps = []
    for r in range(R):
        rows = slice(r * SLAB, (r + 1) * SLAB)
        bt = np.ascontiguousarray(adj[rows, :].T)          # [N, SLAB] f32
        b8 = np.where(bt != 0.0, one8, np.uint8(0)).view(ml_dtypes.float8_e4m3)
        xt = np.ascontiguousarray(x[rows, :].T)            # [F_IN, SLAB]
        in_maps.append({"b": b8, "xt": xt, **common})
    return in_maps


def _run(inputs, trace=False, **kw):
    if "nc" not in _CACHE:
        _CACHE["nc"] = _build()
    nc = _CACHE["nc"]
    in_maps = _prep_inputs(inputs["x"], inputs["adj"], inputs["W1"],
                           inputs["b1"], inputs["W2"], inputs["b2"])
    res = run_bass_kernel_spmd(nc, in_maps, core_ids=list(range(R)),
                               trace=trace, **kw)
    out = np.concatenate([res.results[r]["out"] for r in range(R)], axis=0)
    return out.astype(np.float32), res


def kernel(**inputs):
    out, _ = _run(inputs, trace=False)
    return out


# revision 25
# speedup vs baseline: 1.2059x; 1.1075x over previous
"""GCN (2-layer, gcn_norm) forward on 8 trn2 NeuronCores.

Math (reference):
    norm = D^-1/2 (A + I) D^-1/2
    h      = relu(norm @ (x @ W1) + b1)
    logits = norm @ (h @ W2) + b2
    out    = log_softmax(logits, axis=1)

Kernel strategy:
  - Never materialize norm:  norm @ v  =  ds * (A_hat @ (ds * v)),  ds = rsqrt(deg).
  - Host stages B = adj.T as per-core column slabs (row slab r of adj, transposed,
    C-contiguous) cast to fp8_e4m3 (0/1 entries are exact). TensorE contracts over
    the partition axis, so B's natural [j, i] layout makes A @ v a clean
    moving-operand matmul with perfect DMA patterns.
  - Each core holds its whole 18.9 MB fp8 slab RESIDENT in SBUF: HBM is read once.
  - 3 TensorE passes over the resident slab, all column-tiled so multiple
    chunk-streams run concurrently in the 128x128 PE array:
      deg    : 4 strips (M=1 ones-vector), hidden under the HBM load
      pass 1 : 4 strips of M=32 ([bf16-hi | bf16-lo] split features)
      pass 2 : 2 strips of M=40 (bf16 features)
    Strip partials are summed by one small fold matmul (f32) per block.
  - Self-loops (the +I) are added exactly via f32 identity-rhs matmuls into the
    same PSUM accumulation group, using the core-local f32 feature chunks.
  - Two tiny AllGathers: w' = ds*x@W1 (f32) and v = (ds*h)@W2 (bf16).
  - log_softmax per 128-row tile after a TensorE transpose back to [node, class].
"""
import numpy as np
import ml_dtypes

from concourse import bacc, mybir, tile
from concourse.bass_utils import run_bass_kernel_spmd

N = 12288
F_IN = 512
HID = 16
CLS = 40
R = 8                 # cores
SLAB = N // R         # 1536 rows per core
P = 128
JC = N // P           # 96 contraction chunks
LC = SLAB // P        # 12 local chunks
NB = SLAB // 512      # 3 i-blocks of 512
F32 = mybir.dt.float32
BF16 = mybir.dt.bfloat16
FP8 = mybir.dt.float8e4

_CACHE = {}


def _build():
    nc = bacc.Bacc("TRN2", target_bir_lowering=False, debug=False,
                   enable_asserts=True, num_devices=R)

    b_in = nc.dram_tensor("b", [N, SLAB], FP8, kind="ExternalInput").ap()
    xt_in = nc.dram_tensor("xt", [F_IN, SLAB], F32, kind="ExternalInput").ap()
    w1_in = nc.dram_tensor("w1", [F_IN, HID], F32, kind="ExternalInput").ap()
    b1_in = nc.dram_tensor("b1v", [HID], F32, kind="ExternalInput").ap()
    w2_in = nc.dram_tensor("w2", [HID, CLS], F32, kind="ExternalInput").ap()
    b2_in = nc.dram_tensor("b2v", [CLS], F32, kind="ExternalInput").ap()
    id_in = nc.dram_tensor("ident", [P, P], F32, kind="ExternalInput").ap()
    foldw_in = nc.dram_tensor("foldw", [P, HID], F32, kind="ExternalInput").ap()
    foldv_in = nc.dram_tensor("foldv", [104, CLS], F32, kind="ExternalInput").ap()
    out_d = nc.dram_tensor("out", [SLAB, CLS], F32, kind="ExternalOutput").ap()

    w_bounce = nc.dram_tensor("w_bounce", [SLAB, HID], F32)
    dsc_bounce = nc.dram_tensor("dsc_bounce", [SLAB], F32)
    dsc_full = nc.dram_tensor("dsc_full", [N], F32, addr_space="Shared")
    w_full = nc.dram_tensor("w_full", [N, HID], F32, addr_space="Shared")
    v_bounce = nc.dram_tensor("v_bounce", [SLAB, CLS], BF16)
    v_full = nc.dram_tensor("v_full", [N, CLS], BF16, addr_space="Shared")

    with tile.TileContext(nc) as tc, \
         tc.tile_pool(name="main", bufs=1) as mp, \
         tc.tile_pool(name="scratch", bufs=1) as scr, \
         tc.tile_pool(name="small2", bufs=2) as sp2, \
         tc.tile_pool(name="chain", bufs=1) as ch:

        # ---- constants / small loads -------------------------------------
        w1_sb = mp.tile([P, F_IN // P, HID], F32, name="w1_sb")
        nc.scalar.dma_start(out=w1_sb[:], in_=w1_in.rearrange("(c p) f -> p c f", p=P))
        w2_sb = mp.tile([HID, CLS], F32, name="w2_sb")
        nc.scalar.dma_start(out=w2_sb[:], in_=w2_in)
        b1c = mp.tile([HID, 1], F32, name="b1c")
        nc.scalar.dma_start(out=b1c[:], in_=b1_in.rearrange("(p o) -> p o", o=1))
        b2c = mp.tile([CLS, 1], F32, name="b2c")
        nc.scalar.dma_start(out=b2c[:], in_=b2_in.rearrange("(p o) -> p o", o=1))
        ident_sb = mp.tile([P, P], F32, name="ident_sb")
        nc.scalar.dma_start(out=ident_sb[:], in_=id_in)
        foldw_sb = mp.tile([P, HID], F32, name="foldw_sb")
        nc.scalar.dma_start(out=foldw_sb[:], in_=foldw_in)
        foldv_sb = mp.tile([104, CLS], F32, name="foldv_sb")
        nc.scalar.dma_start(out=foldv_sb[:], in_=foldv_in)
        ones_mat = mp.tile([P, P], FP8, name="ones_mat")
        nc.vector.memset(ones_mat[:], 1.0)

        # ---- x @ W1 first (PSUM-grouped accumulate over the 4 K-chunks) --
        xw1_sb = mp.tile([P, LC, HID], F32, name="xw1_sb")
        with tc.tile_pool(name="ps_x", bufs=1, space="PSUM") as psxp:
            psxg = [psxp.tile([P, 4, HID], F32, name=f"psxg{g}", tag=f"psxg{g}")
                    for g in range(3)]
            for kc in range(F_IN // P):
                xt_chunk = scr.tile([P, SLAB], F32, name="xt_chunk",
                                    tag=f"xt{kc % 2}")
                nc.gpsimd.dma_start(out=xt_chunk[:], in_=xt_in[kc * P:(kc + 1) * P, :])
                for l in range(LC):
                    nc.tensor.matmul(psxg[l // 4][:, l % 4, :],
                                     xt_chunk[:, l * P:(l + 1) * P],
                                     w1_sb[:, kc, :],
                                     start=(kc == 0 and l % 4 == 0),
                                     stop=(kc == F_IN // P - 1),
                                     skip_group_check=True)
            for g in range(3):
                nc.vector.tensor_copy(xw1_sb[:, 4 * g:4 * g + 4, :], psxg[g][:])
        xw1 = xw1_sb
        nc.gpsimd.dma_start(out=w_bounce.ap().rearrange("(c p) f -> p c f", p=P),
                            in_=xw1[:])
        nc.gpsimd.collective_compute(
            "AllGather", mybir.AluOpType.bypass,
            replica_groups=[list(range(R))],
            ins=[w_bounce[:]], outs=[w_full[:]])
        # ---- resident adjacency slab: 12 tiles so deg can chase the load --
        DG = 8  # chunks per load DMA (1.57 MB each)
        b_tiles = []
        for d in range(JC // DG):
            bt = mp.tile([P, DG, SLAB], FP8, name=f"b_sb{d}", tag=f"b_sb{d}")
            b_tiles.append(bt)
            eng = nc.sync if d % 2 == 0 else nc.scalar
            eng.dma_start(
                out=bt[:],
                in_=b_in[d * DG * P:(d + 1) * DG * P, :]
                    .rearrange("(c p) f -> p c f", p=P))

        def b_slice(jc, sl):
            return b_tiles[jc // DG][:, jc % DG, sl]

        # ---- deg pass (single-stream, keeps PE warm under the load) ------
        with tc.tile_pool(name="ps_d", bufs=1, space="PSUM") as psd:
            psdeg = [psd.tile([P, 512], F32, name=f"psdeg{ib}", tag=f"psdeg{ib}")
                     for ib in range(NB)]
            for jc in range(JC):
                for ib in range(NB):
                    nc.tensor.matmul(
                        psdeg[ib][:],
                        ones_mat[:],
                        b_slice(jc, slice(ib * 512, (ib + 1) * 512)),
                        start=(jc == 0), stop=(jc == JC - 1))

            # ---- ds = sqrt(1 / (deg + 1)) --------------------------------
            dsum = scr.tile([1, SLAB], F32, name="dsum", tag="xt1")
            for ib in range(NB):
                nc.vector.tensor_scalar_add(dsum[0:1, ib * 512:(ib + 1) * 512],
                                            psdeg[ib][0:1, :], 1.0)
            dinv = scr.tile([1, SLAB], F32, name="dinv", tag="scratch2")
            nc.vector.reciprocal(dinv[:], dsum[:])
            ds_row = mp.tile([1, SLAB], F32, name="ds_row")  # long-lived
            nc.scalar.activation(ds_row[:], dinv[:], mybir.ActivationFunctionType.Sqrt)

        ds_col = mp.tile([P, LC], F32, name="ds_col")
        with tc.tile_pool(name="ps_dt", bufs=2, space="PSUM") as psdt:
            for l in range(LC):
                pst_ds = psdt.tile([P, 1], F32, name="pst_ds", tag="pst_ds")
                nc.tensor.transpose(pst_ds[:], ds_row[0:1, l * P:(l + 1) * P],
                                    ident_sb[0:1, 0:1])
                nc.vector.tensor_copy(ds_col[:, l:l + 1], pst_ds[:])

        # ---- local w' chunks (for self-loop matmuls) ---------------------
        wprime = mp.tile([P, LC, HID], F32, name="wprime")
        for l in range(LC):
            nc.vector.tensor_scalar_mul(wprime[:, l, :], xw1[:, l, :],
                                        ds_col[:, l:l + 1])

        # ---- gather ds columns (tiny), then whl = bf16 split of ds * xW1 --
        nc.gpsimd.dma_start(out=dsc_bounce.ap().rearrange("(p c) -> p c", c=LC),
                            in_=ds_col[:])
        nc.gpsimd.collective_compute(
            "AllGather", mybir.AluOpType.bypass,
            replica_groups=[list(range(R))],
            ins=[dsc_bounce[:]], outs=[dsc_full[:]])
        ds_colf = mp.tile([P, R, LC, 1], F32, name="ds_colf")
        nc.gpsimd.dma_start(
            out=ds_colf[:],
            in_=dsc_full.ap().rearrange("(r p c o) -> p r c o", p=P, c=LC, o=1))

        wf_stage = scr.tile([P, R, LC, HID], F32, name="wf_stage", tag="xt0")
        nc.gpsimd.dma_start(out=wf_stage[:],
                          in_=w_full.ap().rearrange("(r c p) f -> p r c f",
                                                    p=P, c=LC))
        wsc = scr.tile([P, R, LC, HID], F32, name="wsc", tag="scratch2")
        nc.vector.tensor_tensor(
            out=wsc[:], in0=wf_stage[:],
            in1=ds_colf[:].to_broadcast([P, R, LC, HID]),
            op=mybir.AluOpType.mult)
        whl = mp.tile([P, R, LC, 2 * HID], BF16, name="whl")  # [hi16 | lo16]
        nc.vector.tensor_copy(whl[:, :, :, 0:HID], wsc[:])
        nc.vector.tensor_tensor(out=whl[:, :, :, HID:2 * HID], in0=wsc[:],
                                in1=whl[:, :, :, 0:HID],
                                op=mybir.AluOpType.subtract)

        dsbcs = []
        for ib in range(NB):
            dsb = mp.tile([CLS, 512], F32, name=f"dsbc{ib}", tag=f"dsbc{ib}")
            nc.gpsimd.partition_broadcast(
                dsb[:], ds_row[0:1, ib * 512:(ib + 1) * 512])
            dsbcs.append(dsb)

        # ---- pass 1 (4 col-strips of M=32, per-block epilogue) -----------
        hpT = scr.tile([HID, SLAB], F32, name="hpT", tag="scratch2")
        with tc.tile_pool(name="ps_1", bufs=2, space="PSUM") as ps1p, \
             tc.tile_pool(name="ps_f", bufs=2, space="PSUM") as psfp:
            for ib in range(NB):
                sl = slice(ib * 512, (ib + 1) * 512)
                ps1 = ps1p.tile([P, 512], F32, name="ps1", tag="ps1")
                for jc in range(JC):
                    s = jc % 4
                    nc.tensor.matmul(ps1[32 * s:32 * s + 32, :],
                                     whl[:, jc // LC, jc % LC, :], b_slice(jc, sl),
                                     start=(jc < 4), stop=False,
                                     tile_position=(0, 32 * s),
                                     skip_group_check=True)
                for s in range(4):
                    nc.tensor.matmul(ps1[0:HID, s * P:(s + 1) * P],
                                     wprime[:, 4 * ib + s, :], ident_sb[:],
                                     start=False, stop=(s == 3),
                                     tile_position=(0, 0),
                                     skip_group_check=True)
                xfold = ch.tile([P, 512], F32, name="xfold", tag="xfold")
                nc.vector.tensor_copy(xfold[:], ps1[:])
                psagg = psfp.tile([HID, 512], F32, name="psagg", tag="psagg")
                nc.tensor.matmul(psagg[:], foldw_sb[:], xfold[:],
                                 start=True, stop=True)
                t1 = ch.tile([HID, 512], F32, name="t1", tag="c1")
                nc.vector.tensor_tensor(out=t1[:], in0=psagg[:],
                                        in1=dsbcs[ib][0:HID, :],
                                        op=mybir.AluOpType.mult)
                h2 = ch.tile([HID, 512], F32, name="h2", tag="c2")
                nc.scalar.activation(h2[:], t1[:], mybir.ActivationFunctionType.Relu,
                                     bias=b1c[:, 0:1], scale=1.0)
                nc.vector.tensor_tensor(out=hpT[:, sl], in0=h2[:],
                                        in1=dsbcs[ib][0:HID, :],
                                        op=mybir.AluOpType.mult)

        # ---- v = (ds*h) @ W2 (local f32 + bf16 for gather) ---------------
        vl_sb = mp.tile([P, LC, CLS], F32, name="vl_sb")
        vl_bf = mp.tile([P, LC, CLS], BF16, name="vl_bf")
        with tc.tile_pool(name="ps_v", bufs=2, space="PSUM") as psvp:
            for l in range(LC):
                psv = psvp.tile([P, CLS], F32, name="psv", tag="psv")
                nc.tensor.matmul(psv[:], hpT[:, l * P:(l + 1) * P], w2_sb[:],
                                 start=True, stop=True)
                nc.vector.tensor_copy(vl_sb[:, l, :], psv[:])
                nc.vector.tensor_copy(vl_bf[:, l, :], psv[:])
        nc.gpsimd.dma_start(out=v_bounce.ap().rearrange("(c p) f -> p c f", p=P),
                          in_=vl_bf[:])
        nc.gpsimd.collective_compute(
            "AllGather", mybir.AluOpType.bypass,
            replica_groups=[list(range(R))],
            ins=[v_bounce[:]], outs=[v_full[:]])

        vhl = mp.tile([P, JC, CLS], BF16, name="vhl")
        nc.gpsimd.dma_start(out=vhl[:],
                          in_=v_full.ap().rearrange("(c p) f -> p c f", p=P))

        # ---- pass 2 (2 col-strips of M=40) + log_softmax ------------------
        with tc.tile_pool(name="ps_2", bufs=2, space="PSUM") as ps2p, \
             tc.tile_pool(name="ps_g", bufs=2, space="PSUM") as psgp, \
             tc.tile_pool(name="ps_t", bufs=2, space="PSUM") as pstp:
            for ib in range(NB):
                sl = slice(ib * 512, (ib + 1) * 512)
                ps2 = ps2p.tile([104, 512], F32, name="ps2", tag="ps2")
                for jc in range(JC):
                    s = jc % 2
                    nc.tensor.matmul(ps2[64 * s:64 * s + CLS, :],
                                     vhl[:, jc, :], b_slice(jc, sl),
                                     start=(jc < 2), stop=False,
                                     tile_position=(0, 64 * s),
                                     skip_group_check=True)
                for s in range(4):
                    nc.tensor.matmul(ps2[0:CLS, s * P:(s + 1) * P],
                                     vl_sb[:, 4 * ib + s, :], ident_sb[:],
                                     start=False, stop=(s == 3),
                                     tile_position=(0, 0),
                                     skip_group_check=True)
                xfold2 = ch.tile([104, 512], F32, name="xfold2", tag="xfold")
                nc.vector.tensor_copy(xfold2[:], ps2[0:104, :])
                psagg2 = psgp.tile([CLS, 512], F32, name="psagg2", tag="psagg2")
                nc.tensor.matmul(psagg2[:], foldv_sb[:], xfold2[:],
                                 start=True, stop=True)
                lT = ch.tile([CLS, 512], F32, name="lT", tag="c1")
                nc.vector.tensor_tensor(out=lT[:], in0=psagg2[:],
                                        in1=dsbcs[ib][:, :],
                                        op=mybir.AluOpType.mult)
                lT2 = ch.tile([CLS, 512], F32, name="lT2", tag="c2")
                nc.vector.tensor_scalar_add(lT2[:], lT[:], b2c[:, 0:1])

                for s in range(4):
                    pst = pstp.tile([P, CLS], F32, name="pst", tag="pst")
                    nc.tensor.transpose(pst[:], lT2[:, s * P:(s + 1) * P],
                                        ident_sb[0:CLS, 0:CLS])
                    nm = sp2.tile([P, 1], F32, name="nm", tag="nm")
                    nc.vector.tensor_reduce(nm[:], pst[:], mybir.AxisListType.X,
                                            mybir.AluOpType.max, negate=True)
                    e_sb = sp2.tile([P, CLS], F32, name="e_sb", tag="e_sb")
                    ssum = sp2.tile([P, 1], F32, name="ssum", tag="ssum")
                    nc.scalar.activation(e_sb[:], pst[:],
                                         mybir.ActivationFunctionType.Exp,
                                         bias=nm[:, 0:1], scale=1.0,
                                         accum_out=ssum[:, 0:1])
                    ls = sp2.tile([P, 1], F32, name="ls", tag="ls")
                    nc.scalar.activation(ls[:], ssum[:],
                                         mybir.ActivationFunctionType.Ln)
                    res = sp2.tile([P, CLS], F32, name="res", tag="res")
                    nc.vector.tensor_scalar(res[:], pst[:], nm[:, 0:1], ls[:, 0:1],
                                            op0=mybir.AluOpType.add,
                                            op1=mybir.AluOpType.subtract)
                    row = (ib * 4 + s) * P
                    nc.sync.dma_start(out=out_d[row:row + P, :], in_=res[:])

    nc.compile()
    return nc


def _prep_inputs(x, adj, W1, b1, W2, b2):
    x = np.ascontiguousarray(np.asarray(x, dtype=np.float32))
    adj = np.asarray(adj, dtype=np.float32)
    one8 = np.float32(1.0).astype(ml_dtypes.float8_e4m3).view(np.uint8)
    ident = np.eye(P, dtype=np.float32)
    pp, mm = np.arange(P)[:, None], np.arange(HID)[None, :]
    foldw = (pp % HID == mm).astype(np.float32)          # [128, 16]
    pp4, mm4 = np.arange(104)[:, None], np.arange(CLS)[None, :]
    foldv = ((pp4 == mm4) | (pp4 == mm4 + 64)).astype(np.float32)  # [104, 40]
    common = {
        "w1": np.ascontiguousarray(np.asarray(W1, np.float32)),
        "b1v": np.ascontiguousarray(np.asarray(b1, np.float32)),
        "w2": np.ascontiguousarray(np.asarray(W2, np.float32)),
        "b2v": np.ascontiguousarray(np.asarray(b2, np.float32)),
        "ident": ident, "foldw": foldw, "foldv": foldv,
    }
    in_maps = []
    for r in range(R):
        rows = slice(r * SLAB, (r + 1) * SLAB)
        bt = np.ascontiguousarray(adj[rows, :].T)          # [N, SLAB] f32
        b8 = np.where(bt != 0.0, one8, np.uint8(0)).view(ml_dtypes.float8_e4m3)
        xt = np.ascontiguousarray(x[rows, :].T)            # [F_IN, SLAB]
        in_maps.append({"b": b8, "xt": xt, **common})
    return in_maps


def _run(inputs, trace=False, **kw):
    if "nc" not in _CACHE:
        _CACHE["nc"] = _build()
    nc = _CACHE["nc"]
    in_maps = _prep_inputs(inputs["x"], inputs["adj"], inputs["W1"],
                           inputs["b1"], inputs["W2"], inputs["b2"])
    res = run_bass_kernel_spmd(nc, in_maps, core_ids=list(range(R)),
                               trace=trace, **kw)
    out = np.concatenate([res.results[r]["out"] for r in range(R)], axis=0)
    return out.astype(np.float32), res


def kernel(**inputs):
    out, _ = _run(inputs, trace=False)
    return out


# revision 26
# speedup vs baseline: 1.2648x; 1.0489x over previous
"""GCN (2-layer, gcn_norm) forward on 8 trn2 NeuronCores.

Math (reference):
    norm = D^-1/2 (A + I) D^-1/2
    h      = relu(norm @ (x @ W1) + b1)
    logits = norm @ (h @ W2) + b2
    out    = log_softmax(logits, axis=1)

Kernel strategy:
  - Never materialize norm:  norm @ v  =  ds * (A_hat @ (ds * v)),  ds = rsqrt(deg).
  - Host stages B = adj.T as per-core column slabs (row slab r of adj, transposed,
    C-contiguous) cast to fp8_e4m3 (0/1 entries are exact). TensorE contracts over
    the partition axis, so B's natural [j, i] layout makes A @ v a clean
    moving-operand matmul with perfect DMA patterns.
  - Each core holds its whole 18.9 MB fp8 slab RESIDENT in SBUF: HBM is read once.
  - 3 TensorE passes over the resident slab, all column-tiled so multiple
    chunk-streams run concurrently in the 128x128 PE array:
      deg    : 4 strips (M=1 ones-vector), hidden under the HBM load
      pass 1 : 4 strips of M=32 ([bf16-hi | bf16-lo] split features)
      pass 2 : 2 strips of M=40 (bf16 features)
    Strip partials are summed by one small fold matmul (f32) per block.
  - Self-loops (the +I) are added exactly via f32 identity-rhs matmuls into the
    same PSUM accumulation group, using the core-local f32 feature chunks.
  - Two tiny AllGathers: w' = ds*x@W1 (f32) and v = (ds*h)@W2 (bf16).
  - log_softmax per 128-row tile after a TensorE transpose back to [node, class].
"""
import numpy as np
import ml_dtypes

from concourse import bacc, mybir, tile
from concourse.bass_utils import run_bass_kernel_spmd

N = 12288
F_IN = 512
HID = 16
CLS = 40
R = 8                 # cores
SLAB = N // R         # 1536 rows per core
P = 128
JC = N // P           # 96 contraction chunks
LC = SLAB // P        # 12 local chunks
NB = SLAB // 512      # 3 i-blocks of 512
F32 = mybir.dt.float32
BF16 = mybir.dt.bfloat16
FP8 = mybir.dt.float8e4

_CACHE = {}


def _build():
    nc = bacc.Bacc("TRN2", target_bir_lowering=False, debug=False,
                   enable_asserts=True, num_devices=R)

    b_in = nc.dram_tensor("b", [N, SLAB], FP8, kind="ExternalInput").ap()
    xt_in = nc.dram_tensor("xt", [F_IN, SLAB], F32, kind="ExternalInput").ap()
    w1_in = nc.dram_tensor("w1", [F_IN, HID], F32, kind="ExternalInput").ap()
    b1_in = nc.dram_tensor("b1v", [HID], F32, kind="ExternalInput").ap()
    w2_in = nc.dram_tensor("w2", [HID, CLS], F32, kind="ExternalInput").ap()
    b2_in = nc.dram_tensor("b2v", [CLS], F32, kind="ExternalInput").ap()
    id_in = nc.dram_tensor("ident", [P, P], F32, kind="ExternalInput").ap()
    foldw_in = nc.dram_tensor("foldw", [P, HID], F32, kind="ExternalInput").ap()
    foldv_in = nc.dram_tensor("foldv", [104, CLS], F32, kind="ExternalInput").ap()
    out_d = nc.dram_tensor("out", [SLAB, CLS], F32, kind="ExternalOutput").ap()

    w_bounce = nc.dram_tensor("w_bounce", [SLAB, HID], F32)
    dsc_bounce = nc.dram_tensor("dsc_bounce", [SLAB], F32)
    dsc_full = nc.dram_tensor("dsc_full", [N], F32, addr_space="Shared")
    w_full = nc.dram_tensor("w_full", [N, HID], F32, addr_space="Shared")
    v_bounce = nc.dram_tensor("v_bounce", [SLAB, CLS], BF16)
    v_full = nc.dram_tensor("v_full", [N, CLS], BF16, addr_space="Shared")

    with tile.TileContext(nc) as tc, \
         tc.tile_pool(name="main", bufs=1) as mp, \
         tc.tile_pool(name="scratch", bufs=1) as scr, \
         tc.tile_pool(name="small2", bufs=2) as sp2, \
         tc.tile_pool(name="chain", bufs=1) as ch:

        # ---- constants / small loads -------------------------------------
        w1_sb = mp.tile([P, F_IN // P, HID], F32, name="w1_sb")
        nc.scalar.dma_start(out=w1_sb[:], in_=w1_in.rearrange("(c p) f -> p c f", p=P))
        w2_sb = mp.tile([HID, CLS], F32, name="w2_sb")
        nc.scalar.dma_start(out=w2_sb[:], in_=w2_in)
        b1c = mp.tile([HID, 1], F32, name="b1c")
        nc.scalar.dma_start(out=b1c[:], in_=b1_in.rearrange("(p o) -> p o", o=1))
        b2c = mp.tile([CLS, 1], F32, name="b2c")
        nc.scalar.dma_start(out=b2c[:], in_=b2_in.rearrange("(p o) -> p o", o=1))
        ident_sb = mp.tile([P, P], F32, name="ident_sb")
        nc.scalar.dma_start(out=ident_sb[:], in_=id_in)
        foldw_sb = mp.tile([P, HID], F32, name="foldw_sb")
        nc.scalar.dma_start(out=foldw_sb[:], in_=foldw_in)
        foldv_sb = mp.tile([104, CLS], F32, name="foldv_sb")
        nc.scalar.dma_start(out=foldv_sb[:], in_=foldv_in)
        ones_mat = mp.tile([P, 2, P], FP8, name="ones_mat")
        nc.vector.memset(ones_mat[:], 1.0)

        # ---- x @ W1 first (PSUM-grouped accumulate over the 4 K-chunks) --
        xw1_sb = mp.tile([P, LC, HID], F32, name="xw1_sb")
        with tc.tile_pool(name="ps_x", bufs=1, space="PSUM") as psxp:
            psxg = [psxp.tile([P, 4, HID], F32, name=f"psxg{g}", tag=f"psxg{g}")
                    for g in range(3)]
            for kc in range(F_IN // P):
                xt_chunk = scr.tile([P, SLAB], F32, name="xt_chunk",
                                    tag=f"xt{kc % 2}")
                nc.gpsimd.dma_start(out=xt_chunk[:], in_=xt_in[kc * P:(kc + 1) * P, :])
                for l in range(LC):
                    nc.tensor.matmul(psxg[l // 4][:, l % 4, :],
                                     xt_chunk[:, l * P:(l + 1) * P],
                                     w1_sb[:, kc, :],
                                     start=(kc == 0 and l % 4 == 0),
                                     stop=(kc == F_IN // P - 1),
                                     skip_group_check=True)
            for g in range(3):
                nc.vector.tensor_copy(xw1_sb[:, 4 * g:4 * g + 4, :], psxg[g][:])
        xw1 = xw1_sb
        nc.gpsimd.dma_start(out=w_bounce.ap().rearrange("(c p) f -> p c f", p=P),
                            in_=xw1[:])
        nc.gpsimd.collective_compute(
            "AllGather", mybir.AluOpType.bypass,
            replica_groups=[list(range(R))],
            ins=[w_bounce[:]], outs=[w_full[:]])
        # ---- resident adjacency slab: 12 tiles so deg can chase the load --
        DG = 8  # chunks per load DMA (1.57 MB each)
        b_tiles = []
        for d in range(JC // DG):
            bt = mp.tile([P, DG, SLAB], FP8, name=f"b_sb{d}", tag=f"b_sb{d}")
            b_tiles.append(bt)
            eng = [nc.sync, nc.scalar, nc.sync, nc.gpsimd][d % 4]
            eng.dma_start(
                out=bt[:],
                in_=b_in[d * DG * P:(d + 1) * DG * P, :]
                    .rearrange("(c p) f -> p c f", p=P))

        def b_slice(jc, sl):
            return b_tiles[jc // DG][:, jc % DG, sl]

        # ---- deg pass (single-stream, keeps PE warm under the load) ------
        with tc.tile_pool(name="ps_d", bufs=1, space="PSUM") as psd:
            psdeg = [psd.tile([P, 512], F32, name=f"psdeg{ib}", tag=f"psdeg{ib}")
                     for ib in range(NB)]
            for jp in range(JC // 2):   # DoubleRow: two j-chunks per matmul
                d, m = (2 * jp) // DG, (2 * jp) % DG
                for ib in range(NB):
                    nc.tensor.matmul(
                        psdeg[ib][:],
                        ones_mat[:],
                        b_tiles[d][:, m:m + 2, ib * 512:(ib + 1) * 512],
                        start=(jp == 0), stop=(jp == JC // 2 - 1),
                        perf_mode=mybir.MatmulPerfMode.DoubleRow)

            # ---- ds = sqrt(1 / (deg + 1)) --------------------------------
            dsum = scr.tile([1, SLAB], F32, name="dsum", tag="xt1")
            for ib in range(NB):
                nc.vector.tensor_scalar_add(dsum[0:1, ib * 512:(ib + 1) * 512],
                                            psdeg[ib][0:1, :], 1.0)
            dinv = scr.tile([1, SLAB], F32, name="dinv", tag="scratch2")
            nc.vector.reciprocal(dinv[:], dsum[:])
            ds_row = mp.tile([1, SLAB], F32, name="ds_row")  # long-lived
            nc.scalar.activation(ds_row[:], dinv[:], mybir.ActivationFunctionType.Sqrt)

        ds_col = mp.tile([P, LC], F32, name="ds_col")
        with tc.tile_pool(name="ps_dt", bufs=2, space="PSUM") as psdt:
            for l in range(LC):
                pst_ds = psdt.tile([P, 1], F32, name="pst_ds", tag="pst_ds")
                nc.tensor.transpose(pst_ds[:], ds_row[0:1, l * P:(l + 1) * P],
                                    ident_sb[0:1, 0:1])
                nc.vector.tensor_copy(ds_col[:, l:l + 1], pst_ds[:])

        # ---- local w' chunks (for self-loop matmuls) ---------------------
        wprime = mp.tile([P, LC, HID], F32, name="wprime")
        for l in range(LC):
            nc.vector.tensor_scalar_mul(wprime[:, l, :], xw1[:, l, :],
                                        ds_col[:, l:l + 1])

        # ---- gather ds columns (tiny), then whl = bf16 split of ds * xW1 --
        nc.gpsimd.dma_start(out=dsc_bounce.ap().rearrange("(p c) -> p c", c=LC),
                            in_=ds_col[:])
        nc.gpsimd.collective_compute(
            "AllGather", mybir.AluOpType.bypass,
            replica_groups=[list(range(R))],
            ins=[dsc_bounce[:]], outs=[dsc_full[:]])
        ds_colf = mp.tile([P, R, LC, 1], F32, name="ds_colf")
        nc.gpsimd.dma_start(
            out=ds_colf[:],
            in_=dsc_full.ap().rearrange("(r p c o) -> p r c o", p=P, c=LC, o=1))

        wf_stage = scr.tile([P, R, LC, HID], F32, name="wf_stage", tag="xt0")
        nc.gpsimd.dma_start(out=wf_stage[:],
                          in_=w_full.ap().rearrange("(r c p) f -> p r c f",
                                                    p=P, c=LC))
        wsc = scr.tile([P, R, LC, HID], F32, name="wsc", tag="scratch2")
        nc.vector.tensor_tensor(
            out=wsc[:], in0=wf_stage[:],
            in1=ds_colf[:].to_broadcast([P, R, LC, HID]),
            op=mybir.AluOpType.mult)
        whl = mp.tile([P, R, LC, 2 * HID], BF16, name="whl")  # [hi16 | lo16]
        nc.vector.tensor_copy(whl[:, :, :, 0:HID], wsc[:])
        nc.vector.tensor_tensor(out=whl[:, :, :, HID:2 * HID], in0=wsc[:],
                                in1=whl[:, :, :, 0:HID],
                                op=mybir.AluOpType.subtract)

        dsbcs = []
        for ib in range(NB):
            dsb = mp.tile([CLS, 512], F32, name=f"dsbc{ib}", tag=f"dsbc{ib}")
            nc.gpsimd.partition_broadcast(
                dsb[:], ds_row[0:1, ib * 512:(ib + 1) * 512])
            dsbcs.append(dsb)

        # ---- pass 1 (4 col-strips of M=32, per-block epilogue) -----------
        hpT = scr.tile([HID, SLAB], F32, name="hpT", tag="scratch2")
        with tc.tile_pool(name="ps_1", bufs=2, space="PSUM") as ps1p, \
             tc.tile_pool(name="ps_f", bufs=2, space="PSUM") as psfp:
            for ib in range(NB):
                sl = slice(ib * 512, (ib + 1) * 512)
                ps1 = ps1p.tile([P, 512], F32, name="ps1", tag="ps1")
                for jc in range(JC):
                    s = jc % 4
                    nc.tensor.matmul(ps1[32 * s:32 * s + 32, :],
                                     whl[:, jc // LC, jc % LC, :], b_slice(jc, sl),
                                     start=(jc < 4), stop=False,
                                     tile_position=(0, 32 * s),
                                     skip_group_check=True)
                for s in range(4):
                    nc.tensor.matmul(ps1[0:HID, s * P:(s + 1) * P],
                                     wprime[:, 4 * ib + s, :], ident_sb[:],
                                     start=False, stop=(s == 3),
                                     tile_position=(0, 0),
                                     skip_group_check=True)
                xfold = ch.tile([P, 512], F32, name="xfold", tag="xfold")
                nc.vector.tensor_copy(xfold[:], ps1[:])
                psagg = psfp.tile([HID, 512], F32, name="psagg", tag="psagg")
                nc.tensor.matmul(psagg[:], foldw_sb[:], xfold[:],
                                 start=True, stop=True)
                t1 = ch.tile([HID, 512], F32, name="t1", tag="c1")
                nc.vector.tensor_tensor(out=t1[:], in0=psagg[:],
                                        in1=dsbcs[ib][0:HID, :],
                                        op=mybir.AluOpType.mult)
                h2 = ch.tile([HID, 512], F32, name="h2", tag="c2")
                nc.scalar.activation(h2[:], t1[:], mybir.ActivationFunctionType.Relu,
                                     bias=b1c[:, 0:1], scale=1.0)
                nc.vector.tensor_tensor(out=hpT[:, sl], in0=h2[:],
                                        in1=dsbcs[ib][0:HID, :],
                                        op=mybir.AluOpType.mult)

        # ---- v = (ds*h) @ W2 (local f32 + bf16 for gather) ---------------
        vl_sb = mp.tile([P, LC, CLS], F32, name="vl_sb")
        vl_bf = mp.tile([P, LC, CLS], BF16, name="vl_bf")
        with tc.tile_pool(name="ps_v", bufs=2, space="PSUM") as psvp:
            for l in range(LC):
                psv = psvp.tile([P, CLS], F32, name="psv", tag="psv")
                nc.tensor.matmul(psv[:], hpT[:, l * P:(l + 1) * P], w2_sb[:],
                                 start=True, stop=True)
                nc.vector.tensor_copy(vl_sb[:, l, :], psv[:])
                nc.vector.tensor_copy(vl_bf[:, l, :], psv[:])
        nc.gpsimd.dma_start(out=v_bounce.ap().rearrange("(c p) f -> p c f", p=P),
                          in_=vl_bf[:])
        nc.gpsimd.collective_compute(
            "AllGather", mybir.AluOpType.bypass,
            replica_groups=[list(range(R))],
            ins=[v_bounce[:]], outs=[v_full[:]])

        vhl = mp.tile([P, JC, CLS], BF16, name="vhl")
        nc.gpsimd.dma_start(out=vhl[:],
                          in_=v_full.ap().rearrange("(c p) f -> p c f", p=P))

        # ---- pass 2 (2 col-strips of M=40) + log_softmax ------------------
        with tc.tile_pool(name="ps_2", bufs=2, space="PSUM") as ps2p, \
             tc.tile_pool(name="ps_g", bufs=2, space="PSUM") as psgp, \
             tc.tile_pool(name="ps_t", bufs=2, space="PSUM") as pstp:
            for ib in range(NB):
                sl = slice(ib * 512, (ib + 1) * 512)
                ps2 = ps2p.tile([104, 512], F32, name="ps2", tag="ps2")
                for jc in range(JC):
                    s = jc % 2
                    nc.tensor.matmul(ps2[64 * s:64 * s + CLS, :],
                                     vhl[:, jc, :], b_slice(jc, sl),
                                     start=(jc < 2), stop=False,
                                     tile_position=(0, 64 * s),
                                     skip_group_check=True)
                for s in range(4):
                    nc.tensor.matmul(ps2[0:CLS, s * P:(s + 1) * P],
                                     vl_sb[:, 4 * ib + s, :], ident_sb[:],
                                     start=False, stop=(s == 3),
                                     tile_position=(0, 0),
                                     skip_group_check=True)
                xfold2 = ch.tile([104, 512], F32, name="xfold2", tag="xfold")
                nc.vector.tensor_copy(xfold2[:], ps2[0:104, :])
                psagg2 = psgp.tile([CLS, 512], F32, name="psagg2", tag="psagg2")
                nc.tensor.matmul(psagg2[:], foldv_sb[:], xfold2[:],
                                 start=True, stop=True)
                lT = ch.tile([CLS, 512], F32, name="lT", tag="c1")
                nc.vector.tensor_tensor(out=lT[:], in0=psagg2[:],
                                        in1=dsbcs[ib][:, :],
                                        op=mybir.AluOpType.mult)
                lT2 = ch.tile([CLS, 512], F32, name="lT2", tag="c2")
                nc.vector.tensor_scalar_add(lT2[:], lT[:], b2c[:, 0:1])

                for s in range(4):
                    pst = pstp.tile([P, CLS], F32, name="pst", tag="pst")
                    nc.tensor.transpose(pst[:], lT2[:, s * P:(s + 1) * P],
                                        ident_sb[0:CLS, 0:CLS])
                    nm = sp2.tile([P, 1], F32, name="nm", tag="nm")
                    nc.vector.tensor_reduce(nm[:], pst[:], mybir.AxisListType.X,
                                            mybir.AluOpType.max, negate=True)
                    e_sb = sp2.tile([P, CLS], F32, name="e_sb", tag="e_sb")
                    ssum = sp2.tile([P, 1], F32, name="ssum", tag="ssum")
                    nc.scalar.activation(e_sb[:], pst[:],
                                         mybir.ActivationFunctionType.Exp,
                                         bias=nm[:, 0:1], scale=1.0,
                                         accum_out=ssum[:, 0:1])
                    ls = sp2.tile([P, 1], F32, name="ls", tag="ls")
                    nc.scalar.activation(ls[:], ssum[:],
                                         mybir.ActivationFunctionType.Ln)
                    res = sp2.tile([P, CLS], F32, name="res", tag="res")
                    nc.vector.tensor_scalar(res[:], pst[:], nm[:, 0:1], ls[:, 0:1],
                                            op0=mybir.AluOpType.add,
                                            op1=mybir.AluOpType.subtract)
                    row = (ib * 4 + s) * P
                    nc.sync.dma_start(out=out_d[row:row + P, :], in_=res[:])

    nc.compile()
    return nc


def _prep_inputs(x, adj, W1, b1, W2, b2):
    x = np.ascontiguousarray(np.asarray(x, dtype=np.float32))
    adj = np.asarray(adj, dtype=np.float32)
    one8 = np.float32(1.0).astype(ml_dtypes.float8_e4m3).view(np.uint8)
    ident = np.eye(P, dtype=np.float32)
    pp, mm = np.arange(P)[:, None], np.arange(HID)[None, :]
    foldw = (pp % HID == mm).astype(np.float32)          # [128, 16]
    pp4, mm4 = np.arange(104)[:, None], np.arange(CLS)[None, :]
    foldv = ((pp4 == mm4) | (pp4 == mm4 + 64)).astype(np.float32)  # [104, 40]
    common = {
        "w1": np.ascontiguousarray(np.asarray(W1, np.float32)),
        "b1v": np.ascontiguousarray(np.asarray(b1, np.float32)),
        "w2": np.ascontiguousarray(np.asarray(W2, np.float32)),
        "b2v": np.ascontiguousarray(np.asarray(b2, np.float32)),
        "ident": ident, "foldw": foldw, "foldv": foldv,
    }
    in_maps = []
    for r in range(R):
        rows = slice(r * SLAB, (r + 1) * SLAB)
        bt = np.ascontiguousarray(adj[rows, :].T)          # [N, SLAB] f32
        b8 = np.where(bt != 0.0, one8, np.uint8(0)).view(ml_dtypes.float8_e4m3)
        xt = np.ascontiguousarray(x[rows, :].T)            # [F_IN, SLAB]
        in_maps.append({"b": b8, "xt": xt, **common})
    return in_maps


def _run(inputs, trace=False, **kw):
    if "nc" not in _CACHE:
        _CACHE["nc"] = _build()
    nc = _CACHE["nc"]
    in_maps = _prep_inputs(inputs["x"], inputs["adj"], inputs["W1"],
                           inputs["b1"], inputs["W2"], inputs["b2"])
    res = run_bass_kernel_spmd(nc, in_maps, core_ids=list(range(R)),
                               trace=trace, **kw)
    out = np.concatenate([res.results[r]["out"] for r in range(R)], axis=0)
    return out.astype(np.float32), res


def kernel(**inputs):
    out, _ = _run(inputs, trace=False)
    return out


# revision 27
# speedup vs baseline: 1.3119x; 1.0372x over previous
"""GCN (2-layer, gcn_norm) forward on 8 trn2 NeuronCores.

Math (reference):
    norm = D^-1/2 (A + I) D^-1/2
    h      = relu(norm @ (x @ W1) + b1)
    logits = norm @ (h @ W2) + b2
    out    = log_softmax(logits, axis=1)

Kernel strategy:
  - Never materialize norm:  norm @ v  =  ds * (A_hat @ (ds * v)),  ds = rsqrt(deg).
  - Host stages B = adj.T as per-core column slabs (row slab r of adj, transposed,
    C-contiguous) cast to fp8_e4m3 (0/1 entries are exact). TensorE contracts over
    the partition axis, so B's natural [j, i] layout makes A @ v a clean
    moving-operand matmul with perfect DMA patterns.
  - Each core holds its whole 18.9 MB fp8 slab RESIDENT in SBUF: HBM is read once.
  - 3 TensorE passes over the resident slab, all column-tiled so multiple
    chunk-streams run concurrently in the 128x128 PE array:
      deg    : 4 strips (M=1 ones-vector), hidden under the HBM load
      pass 1 : 4 strips of M=32 ([bf16-hi | bf16-lo] split features)
      pass 2 : 2 strips of M=40 (bf16 features)
    Strip partials are summed by one small fold matmul (f32) per block.
  - Self-loops (the +I) are added exactly via f32 identity-rhs matmuls into the
    same PSUM accumulation group, using the core-local f32 feature chunks.
  - Two tiny AllGathers: w' = ds*x@W1 (f32) and v = (ds*h)@W2 (bf16).
  - log_softmax per 128-row tile after a TensorE transpose back to [node, class].
"""
import numpy as np
import ml_dtypes

from concourse import bacc, mybir, tile
from concourse.bass_utils import run_bass_kernel_spmd

N = 12288
F_IN = 512
HID = 16
CLS = 40
R = 8                 # cores
SLAB = N // R         # 1536 rows per core
P = 128
JC = N // P           # 96 contraction chunks
LC = SLAB // P        # 12 local chunks
NB = SLAB // 512      # 3 i-blocks of 512
F32 = mybir.dt.float32
BF16 = mybir.dt.bfloat16
FP8 = mybir.dt.float8e4

_CACHE = {}


def _build():
    nc = bacc.Bacc("TRN2", target_bir_lowering=False, debug=False,
                   enable_asserts=True, num_devices=R)

    b_in = nc.dram_tensor("b", [N, SLAB], FP8, kind="ExternalInput").ap()
    xt_in = nc.dram_tensor("xt", [F_IN, SLAB], F32, kind="ExternalInput").ap()
    w1_in = nc.dram_tensor("w1", [F_IN, HID], F32, kind="ExternalInput").ap()
    b1_in = nc.dram_tensor("b1v", [HID], F32, kind="ExternalInput").ap()
    w2_in = nc.dram_tensor("w2", [HID, CLS], F32, kind="ExternalInput").ap()
    b2_in = nc.dram_tensor("b2v", [CLS], F32, kind="ExternalInput").ap()
    id_in = nc.dram_tensor("ident", [P, P], F32, kind="ExternalInput").ap()
    foldw_in = nc.dram_tensor("foldw", [P, HID], F32, kind="ExternalInput").ap()
    foldv_in = nc.dram_tensor("foldv", [104, CLS], F32, kind="ExternalInput").ap()
    out_d = nc.dram_tensor("out", [SLAB, CLS], F32, kind="ExternalOutput").ap()

    w_bounce = nc.dram_tensor("w_bounce", [SLAB, HID], F32)
    dsc_bounce = nc.dram_tensor("dsc_bounce", [SLAB], F32)
    dsc_full = nc.dram_tensor("dsc_full", [N], F32, addr_space="Shared")
    w_full = nc.dram_tensor("w_full", [N, HID], F32, addr_space="Shared")
    v_bounce = nc.dram_tensor("v_bounce", [SLAB, CLS], BF16)
    v_full = nc.dram_tensor("v_full", [N, CLS], BF16, addr_space="Shared")

    with tile.TileContext(nc) as tc, \
         tc.tile_pool(name="main", bufs=1) as mp, \
         tc.tile_pool(name="scratch", bufs=1) as scr, \
         tc.tile_pool(name="small2", bufs=2) as sp2, \
         tc.tile_pool(name="chain", bufs=1) as ch:

        # ---- constants / small loads -------------------------------------
        w1_sb = mp.tile([P, F_IN // P, HID], F32, name="w1_sb")
        nc.scalar.dma_start(out=w1_sb[:], in_=w1_in.rearrange("(c p) f -> p c f", p=P))
        w2_sb = mp.tile([HID, CLS], F32, name="w2_sb")
        nc.scalar.dma_start(out=w2_sb[:], in_=w2_in)
        b1c = mp.tile([HID, 1], F32, name="b1c")
        nc.scalar.dma_start(out=b1c[:], in_=b1_in.rearrange("(p o) -> p o", o=1))
        b2c = mp.tile([CLS, 1], F32, name="b2c")
        nc.scalar.dma_start(out=b2c[:], in_=b2_in.rearrange("(p o) -> p o", o=1))
        ident_sb = mp.tile([P, P], F32, name="ident_sb")
        nc.scalar.dma_start(out=ident_sb[:], in_=id_in)
        foldw_sb = mp.tile([P, HID], F32, name="foldw_sb")
        nc.scalar.dma_start(out=foldw_sb[:], in_=foldw_in)
        foldv_sb = mp.tile([104, CLS], F32, name="foldv_sb")
        nc.scalar.dma_start(out=foldv_sb[:], in_=foldv_in)
        ones_mat = mp.tile([P, 2, P], FP8, name="ones_mat")
        nc.vector.memset(ones_mat[:], 1.0)

        # ---- x @ W1 first (PSUM-grouped accumulate over the 4 K-chunks) --
        xw1_sb = mp.tile([P, LC, HID], F32, name="xw1_sb")
        with tc.tile_pool(name="ps_x", bufs=1, space="PSUM") as psxp:
            psxg = [psxp.tile([P, 4, HID], F32, name=f"psxg{g}", tag=f"psxg{g}")
                    for g in range(3)]
            for kc in range(F_IN // P):
                xt_chunk = scr.tile([P, SLAB], F32, name="xt_chunk",
                                    tag=f"xt{kc % 2}")
                nc.gpsimd.dma_start(out=xt_chunk[:], in_=xt_in[kc * P:(kc + 1) * P, :])
                for l in range(LC):
                    nc.tensor.matmul(psxg[l // 4][:, l % 4, :],
                                     xt_chunk[:, l * P:(l + 1) * P],
                                     w1_sb[:, kc, :],
                                     start=(kc == 0 and l % 4 == 0),
                                     stop=(kc == F_IN // P - 1),
                                     skip_group_check=True)
            for g in range(3):
                nc.vector.tensor_copy(xw1_sb[:, 4 * g:4 * g + 4, :], psxg[g][:])
        xw1 = xw1_sb
        nc.gpsimd.dma_start(out=w_bounce.ap().rearrange("(c p) f -> p c f", p=P),
                            in_=xw1[:])
        nc.gpsimd.collective_compute(
            "AllGather", mybir.AluOpType.bypass,
            replica_groups=[list(range(R))],
            ins=[w_bounce[:]], outs=[w_full[:]])
        # ---- resident adjacency slab: 12 tiles so deg can chase the load --
        DG = 8  # chunks per load DMA (1.57 MB each)
        b_tiles = []
        for d in range(JC // DG):
            bt = mp.tile([P, DG, SLAB], FP8, name=f"b_sb{d}", tag=f"b_sb{d}")
            b_tiles.append(bt)
            eng = nc.sync
            eng.dma_start(
                out=bt[:],
                in_=b_in[d * DG * P:(d + 1) * DG * P, :]
                    .rearrange("(c p) f -> p c f", p=P))

        def b_slice(jc, sl):
            return b_tiles[jc // DG][:, jc % DG, sl]

        # ---- deg pass (single-stream, keeps PE warm under the load) ------
        with tc.tile_pool(name="ps_d", bufs=1, space="PSUM") as psd:
            psdeg = [psd.tile([P, 512], F32, name=f"psdeg{ib}", tag=f"psdeg{ib}")
                     for ib in range(NB)]
            for jp in range(JC // 2):   # DoubleRow: two j-chunks per matmul
                d, m = (2 * jp) // DG, (2 * jp) % DG
                for ib in range(NB):
                    nc.tensor.matmul(
                        psdeg[ib][:],
                        ones_mat[:],
                        b_tiles[d][:, m:m + 2, ib * 512:(ib + 1) * 512],
                        start=(jp == 0), stop=(jp == JC // 2 - 1),
                        perf_mode=mybir.MatmulPerfMode.DoubleRow)

            # ---- ds = sqrt(1 / (deg + 1)) --------------------------------
            dsum = scr.tile([1, SLAB], F32, name="dsum", tag="xt1")
            for ib in range(NB):
                nc.vector.tensor_scalar_add(dsum[0:1, ib * 512:(ib + 1) * 512],
                                            psdeg[ib][0:1, :], 1.0)
            dinv = scr.tile([1, SLAB], F32, name="dinv", tag="scratch2")
            nc.vector.reciprocal(dinv[:], dsum[:])
            ds_row = mp.tile([1, SLAB], F32, name="ds_row")  # long-lived
            nc.scalar.activation(ds_row[:], dinv[:], mybir.ActivationFunctionType.Sqrt)

        ds_col = mp.tile([P, LC], F32, name="ds_col")
        with tc.tile_pool(name="ps_dt", bufs=2, space="PSUM") as psdt:
            for l in range(LC):
                pst_ds = psdt.tile([P, 1], F32, name="pst_ds", tag="pst_ds")
                nc.tensor.transpose(pst_ds[:], ds_row[0:1, l * P:(l + 1) * P],
                                    ident_sb[0:1, 0:1])
                nc.vector.tensor_copy(ds_col[:, l:l + 1], pst_ds[:])

        # ---- local w' chunks (for self-loop matmuls) ---------------------
        wprime = mp.tile([P, LC, HID], F32, name="wprime")
        for l in range(LC):
            nc.vector.tensor_scalar_mul(wprime[:, l, :], xw1[:, l, :],
                                        ds_col[:, l:l + 1])

        # ---- gather ds columns (tiny), then whl = bf16 split of ds * xW1 --
        nc.gpsimd.dma_start(out=dsc_bounce.ap().rearrange("(p c) -> p c", c=LC),
                            in_=ds_col[:])
        nc.gpsimd.collective_compute(
            "AllGather", mybir.AluOpType.bypass,
            replica_groups=[list(range(R))],
            ins=[dsc_bounce[:]], outs=[dsc_full[:]])
        ds_colf = mp.tile([P, R, LC, 1], F32, name="ds_colf")
        nc.gpsimd.dma_start(
            out=ds_colf[:],
            in_=dsc_full.ap().rearrange("(r p c o) -> p r c o", p=P, c=LC, o=1))

        wf_stage = scr.tile([P, R, LC, HID], F32, name="wf_stage", tag="xt0")
        nc.gpsimd.dma_start(out=wf_stage[:],
                          in_=w_full.ap().rearrange("(r c p) f -> p r c f",
                                                    p=P, c=LC))
        wsc = scr.tile([P, R, LC, HID], F32, name="wsc", tag="scratch2")
        nc.vector.tensor_tensor(
            out=wsc[:], in0=wf_stage[:],
            in1=ds_colf[:].to_broadcast([P, R, LC, HID]),
            op=mybir.AluOpType.mult)
        whl = mp.tile([P, R, LC, 2 * HID], BF16, name="whl")  # [hi16 | lo16]
        nc.vector.tensor_copy(whl[:, :, :, 0:HID], wsc[:])
        nc.vector.tensor_tensor(out=whl[:, :, :, HID:2 * HID], in0=wsc[:],
                                in1=whl[:, :, :, 0:HID],
                                op=mybir.AluOpType.subtract)

        dsbcs = []
        for ib in range(NB):
            dsb = mp.tile([CLS, 512], F32, name=f"dsbc{ib}", tag=f"dsbc{ib}")
            nc.gpsimd.partition_broadcast(
                dsb[:], ds_row[0:1, ib * 512:(ib + 1) * 512])
            dsbcs.append(dsb)

        # ---- pass 1 (4 col-strips of M=32, per-block epilogue) -----------
        hpT = scr.tile([HID, SLAB], F32, name="hpT", tag="scratch2")
        with tc.tile_pool(name="ps_1", bufs=2, space="PSUM") as ps1p, \
             tc.tile_pool(name="ps_f", bufs=2, space="PSUM") as psfp:
            for ib in range(NB):
                sl = slice(ib * 512, (ib + 1) * 512)
                ps1 = ps1p.tile([P, 512], F32, name="ps1", tag="ps1")
                for jc in range(JC):
                    s = jc % 4
                    nc.tensor.matmul(ps1[32 * s:32 * s + 32, :],
                                     whl[:, jc // LC, jc % LC, :], b_slice(jc, sl),
                                     start=(jc < 4), stop=False,
                                     tile_position=(0, 32 * s),
                                     skip_group_check=True)
                for s in range(4):
                    nc.tensor.matmul(ps1[0:HID, s * P:(s + 1) * P],
                                     wprime[:, 4 * ib + s, :], ident_sb[:],
                                     start=False, stop=(s == 3),
                                     tile_position=(0, 0),
                                     skip_group_check=True)
                xfold = ch.tile([P, 512], F32, name="xfold", tag="xfold")
                nc.vector.tensor_copy(xfold[:], ps1[:])
                psagg = psfp.tile([HID, 512], F32, name="psagg", tag="psagg")
                nc.tensor.matmul(psagg[:], foldw_sb[:], xfold[:],
                                 start=True, stop=True)
                t1 = ch.tile([HID, 512], F32, name="t1", tag="c1")
                nc.vector.tensor_tensor(out=t1[:], in0=psagg[:],
                                        in1=dsbcs[ib][0:HID, :],
                                        op=mybir.AluOpType.mult)
                h2 = ch.tile([HID, 512], F32, name="h2", tag="c2")
                nc.scalar.activation(h2[:], t1[:], mybir.ActivationFunctionType.Relu,
                                     bias=b1c[:, 0:1], scale=1.0)
                nc.vector.tensor_tensor(out=hpT[:, sl], in0=h2[:],
                                        in1=dsbcs[ib][0:HID, :],
                                        op=mybir.AluOpType.mult)

        # ---- v = (ds*h) @ W2 (local f32 + bf16 for gather) ---------------
        vl_sb = mp.tile([P, LC, CLS], F32, name="vl_sb")
        vl_bf = mp.tile([P, LC, CLS], BF16, name="vl_bf")
        with tc.tile_pool(name="ps_v", bufs=2, space="PSUM") as psvp:
            for l in range(LC):
                psv = psvp.tile([P, CLS], F32, name="psv", tag="psv")
                nc.tensor.matmul(psv[:], hpT[:, l * P:(l + 1) * P], w2_sb[:],
                                 start=True, stop=True)
                nc.vector.tensor_copy(vl_sb[:, l, :], psv[:])
                nc.vector.tensor_copy(vl_bf[:, l, :], psv[:])
        nc.gpsimd.dma_start(out=v_bounce.ap().rearrange("(c p) f -> p c f", p=P),
                          in_=vl_bf[:])
        nc.gpsimd.collective_compute(
            "AllGather", mybir.AluOpType.bypass,
            replica_groups=[list(range(R))],
            ins=[v_bounce[:]], outs=[v_full[:]])

        vhl = mp.tile([P, JC, CLS], BF16, name="vhl")
        nc.gpsimd.dma_start(out=vhl[:],
                          in_=v_full.ap().rearrange("(c p) f -> p c f", p=P))

        # ---- pass 2 (2 col-strips of M=40) + log_softmax ------------------
        with tc.tile_pool(name="ps_2", bufs=2, space="PSUM") as ps2p, \
             tc.tile_pool(name="ps_g", bufs=2, space="PSUM") as psgp, \
             tc.tile_pool(name="ps_t", bufs=2, space="PSUM") as pstp:
            for ib in range(NB):
                sl = slice(ib * 512, (ib + 1) * 512)
                ps2 = ps2p.tile([104, 512], F32, name="ps2", tag="ps2")
                for jc in range(JC):
                    s = jc % 2
                    nc.tensor.matmul(ps2[64 * s:64 * s + CLS, :],
                                     vhl[:, jc, :], b_slice(jc, sl),
                                     start=(jc < 2), stop=False,
                                     tile_position=(0, 64 * s),
                                     skip_group_check=True)
                for s in range(4):
                    nc.tensor.matmul(ps2[0:CLS, s * P:(s + 1) * P],
                                     vl_sb[:, 4 * ib + s, :], ident_sb[:],
                                     start=False, stop=(s == 3),
                                     tile_position=(0, 0),
                                     skip_group_check=True)
                xfold2 = ch.tile([104, 512], F32, name="xfold2", tag="xfold")
                nc.vector.tensor_copy(xfold2[:], ps2[0:104, :])
                psagg2 = psgp.tile([CLS, 512], F32, name="psagg2", tag="psagg2")
                nc.tensor.matmul(psagg2[:], foldv_sb[:], xfold2[:],
                                 start=True, stop=True)
                lT = ch.tile([CLS, 512], F32, name="lT", tag="c1")
                nc.vector.tensor_tensor(out=lT[:], in0=psagg2[:],
                                        in1=dsbcs[ib][:, :],
                                        op=mybir.AluOpType.mult)
                lT2 = ch.tile([CLS, 512], F32, name="lT2", tag="c2")
                nc.vector.tensor_scalar_add(lT2[:], lT[:], b2c[:, 0:1])

                for s in range(4):
                    pst = pstp.tile([P, CLS], F32, name="pst", tag="pst")
                    nc.tensor.transpose(pst[:], lT2[:, s * P:(s + 1) * P],
                                        ident_sb[0:CLS, 0:CLS])
                    nm = sp2.tile([P, 1], F32, name="nm", tag="nm")
                    nc.vector.tensor_reduce(nm[:], pst[:], mybir.AxisListType.X,
                                            mybir.AluOpType.max, negate=True)
                    e_sb = sp2.tile([P, CLS], F32, name="e_sb", tag="e_sb")
                    ssum = sp2.tile([P, 1], F32, name="ssum", tag="ssum")
                    nc.scalar.activation(e_sb[:], pst[:],
                                         mybir.ActivationFunctionType.Exp,
                                         bias=nm[:, 0:1], scale=1.0,
                                         accum_out=ssum[:, 0:1])
                    ls = sp2.tile([P, 1], F32, name="ls", tag="ls")
                    nc.scalar.activation(ls[:], ssum[:],
                                         mybir.ActivationFunctionType.Ln)
                    res = sp2.tile([P, CLS], F32, name="res", tag="res")
                    nc.vector.tensor_scalar(res[:], pst[:], nm[:, 0:1], ls[:, 0:1],
                                            op0=mybir.AluOpType.add,
                                            op1=mybir.AluOpType.subtract)
                    row = (ib * 4 + s) * P
                    nc.sync.dma_start(out=out_d[row:row + P, :], in_=res[:])

    nc.compile()
    return nc


def _prep_inputs(x, adj, W1, b1, W2, b2):
    x = np.ascontiguousarray(np.asarray(x, dtype=np.float32))
    adj = np.asarray(adj, dtype=np.float32)
    one8 = np.float32(1.0).astype(ml_dtypes.float8_e4m3).view(np.uint8)
    ident = np.eye(P, dtype=np.float32)
    pp, mm = np.arange(P)[:, None], np.arange(HID)[None, :]
    foldw = (pp % HID == mm).astype(np.float32)          # [128, 16]
    pp4, mm4 = np.arange(104)[:, None], np.arange(CLS)[None, :]
    foldv = ((pp4 == mm4) | (pp4 == mm4 + 64)).astype(np.float32)  # [104, 40]
    common = {
        "w1": np.ascontiguousarray(np.asarray(W1, np.float32)),
        "b1v": np.ascontiguousarray(np.asarray(b1, np.float32)),
        "w2": np.ascontiguousarray(np.asarray(W2, np.float32)),
        "b2v": np.ascontiguousarray(np.asarray(b2, np.float32)),
        "ident": ident, "foldw": foldw, "foldv": foldv,
    }
    in_maps = []
    for r in range(R):
        rows = slice(r * SLAB, (r + 1) * SLAB)
        bt = np.ascontiguousarray(adj[rows, :].T)          # [N, SLAB] f32
        b8 = np.where(bt != 0.0, one8, np.uint8(0)).view(ml_dtypes.float8_e4m3)
        xt = np.ascontiguousarray(x[rows, :].T)            # [F_IN, SLAB]
        in_maps.append({"b": b8, "xt": xt, **common})
    return in_maps


def _run(inputs, trace=False, **kw):
    if "nc" not in _CACHE:
        _CACHE["nc"] = _build()
    nc = _CACHE["nc"]
    in_maps = _prep_inputs(inputs["x"], inputs["adj"], inputs["W1"],
                           inputs["b1"], inputs["W2"], inputs["b2"])
    res = run_bass_kernel_spmd(nc, in_maps, core_ids=list(range(R)),
                               trace=trace, **kw)
    out = np.concatenate([res.results[r]["out"] for r in range(R)], axis=0)
    return out.astype(np.float32), res


def kernel(**inputs):
    out, _ = _run(inputs, trace=False)
    return out
